# revision 46
# baseline (speedup 1.0000x reference)
"""Multi-head attention (B=2, S=2048, D=1024, H=16) on 8 trn2 cores.

Sharding: core c handles batch b = c//4 and heads 4g..4g+3 where g = c%4
(tensor-parallel on heads: Wq/Wk/Wv column-sharded, Wpost row-sharded).
Each core emits a partial [S, D] output; host sums the 4 partials per batch
and adds bpost.

v3b pipeline: ACT-paced stream with concurrent tiled matmuls. HW-verified
(microbenchmark + production traces): matmuls whose 32-row/col array strips
are disjoint run concurrently (4ns stagger) when adjacent in the stream.
Per (unit, j) step the PE does ~5 slot-times instead of 8:
  - scores: 2 slots. Pair = (kv tile 2j, kv tile 2j+1) of the SAME head on
    opposite K=64 row halves; the i=1 member reads a row-swapped duplicate
    of kT/qT (built by GpSimd copies) so both members share one exp/PSUM
    dependency and the scheduler keeps them adjacent.
  - AV: 2 slots (both heads via M=64 col halves, no ones row).
  - softmax denominators: 1 slot (quad of M=1 ones-contractions into spare
    PSUM rows of the AV accumulator banks).
The exp stream on ACT (2 x 1107ns per step) is the pacer; projections, v
tiles and post ride the PE slack. Post outputs DMA straight from PSUM.
PE is pre-warmed with dummy matmuls so the HAM clock gate opens before
real data lands; biases ride as 2 leading columns of the wq/wk pages.
"""

import os

import numpy as np
import ml_dtypes

import concourse.bass as bass
import concourse.tile as tile
from concourse import bacc
from concourse import mybir
from concourse.bass_utils import run_bass_kernel_spmd

F32 = mybir.dt.float32
BF16 = mybir.dt.bfloat16

B, S, D, H = 2, 2048, 1024, 16
DK = D // H          # 64
HPC = 4              # heads per core
DCORE = HPC * DK     # 256 output dims per core
NKT = D // 128       # 8 contraction tiles over d_in
NMT = S // 128       # 16 token tiles
QB = 512             # query block
NQB = S // QB        # 4
NKV = S // 128       # 16 kv tiles
NJ = NKV // 2        # 8 kv-pair chunks per unit
XBW = NKT * QB       # 4096 packed x columns per 512-token block
WQW = 2 + 2 * NKT * 128   # wq/wk page: 2 bias cols + p-major kt tiles

_CACHE = {}
LAST_RESULTS = None


def _ensure_ntff_hook():
    """The agent image's antenv lacks axon_hooks; synthesize it and register
    the ctypes NTFF profiling hook so trace=True yields exec times."""
    import sys
    import types

    try:
        from antenv import axon_hooks  # noqa: F401
        return
    except ImportError:
        pass
    mod = types.ModuleType("antenv.axon_hooks")
    _state = {"hook": None}
    mod.set_axon_ntff_profile_hook = lambda h: _state.__setitem__("hook", h)
    mod.get_axon_ntff_profile_hook = lambda: _state["hook"]
    sys.modules["antenv.axon_hooks"] = mod
    import antenv

    antenv.axon_hooks = mod
    try:
        import trn_agent_boot.trn_boot as _tb

        hook = _tb._ntff_profile_via_ctypes("/opt/axon/libaxon_pjrt.so")
        mod.set_axon_ntff_profile_hook(hook)
    except Exception:
        pass


def _build(with_mask: bool):
    nc = bacc.Bacc(None, target_bir_lowering=False)

    xq_d = nc.declare_dram_parameter("xq", [128, NQB * XBW], BF16, isOutput=False)
    xk_d = nc.declare_dram_parameter("xk", [128, NQB * XBW], BF16, isOutput=False)
    xv_d = nc.declare_dram_parameter("xv", [128, NQB * XBW], BF16, isOutput=False)
    wq_d = nc.declare_dram_parameter("wq", [128, WQW], BF16, isOutput=False)
    wk_d = nc.declare_dram_parameter("wk", [128, WQW], BF16, isOutput=False)
    wv_d = nc.declare_dram_parameter("wv", [128, NKT * DCORE], BF16, isOutput=False)
    wp_d = nc.declare_dram_parameter("wp", [128, 2 * D], BF16, isOutput=False)
    maskT = None
    if with_mask:
        maskT = nc.declare_dram_parameter("maskT", [S, S], F32, isOutput=False)
    out_d = nc.declare_dram_parameter("out_p", [S, D], BF16, isOutput=True)

    with tile.TileContext(nc) as tc:
        with (
            tc.tile_pool(name="persist", bufs=1) as persist,
            tc.tile_pool(name="wpool", bufs=1) as wpool,
            tc.tile_pool(name="xkp", bufs=2) as xkp,
            tc.tile_pool(name="xqp", bufs=2) as xqp,
            tc.tile_pool(name="xvp", bufs=3) as xvp,
            tc.tile_pool(name="sexp", bufs=16) as sexp,
            tc.tile_pool(name="small", bufs=2) as small,
            tc.tile_pool(name="outs", bufs=2) as outs,
            tc.tile_pool(name="mpool", bufs=2) as mpool,
            tc.tile_pool(name="pss", bufs=1, space="PSUM") as pss,
            tc.tile_pool(name="pso", bufs=1, space="PSUM") as pso,
            tc.tile_pool(name="mix", bufs=2, space="PSUM") as mix,
        ):
            # ---- PE pre-warm (HAM clock gate) + ACT table warm ----
            warm_sb = persist.tile([128, 512], BF16, tag="warm_sb", name="warm_sb")
            nc.vector.memset(warm_sb, 0.01)
            ones_col = persist.tile([128, 1], BF16, tag="ones", name="ones")
            nc.vector.memset(ones_col, 1.0)
            warm_act = small.tile([1, 128], F32, tag="warm", name="warm")
            nc.scalar.activation(
                out=warm_act, in_=warm_sb[0:1, 0:128],
                func=mybir.ActivationFunctionType.Exp,
            )
            warm_ps = mix.tile([128, 512], F32, tag="mix", name="warmps")
            for _ in range(22):
                nc.tensor.matmul(
                    warm_ps, warm_sb[:, 0:128], warm_sb[:, :],
                    start=True, stop=True, skip_group_check=True,
                )

            # ---- input DMAs in deadline order ----
            xk_t = {}
            xq_t = {}
            xv_t = {}

            def dma_x(store, pool, src, nb, tag, split=1, eng=None):
                t = pool.tile([128, XBW], BF16, tag=tag, name=f"{tag}{nb}")
                hw = XBW // split
                for h in range(split):
                    (eng or nc.sync).dma_start(
                        out=t[:, hw * h : hw * (h + 1)],
                        in_=src[:, XBW * nb + hw * h : XBW * nb + hw * (h + 1)],
                    )
                store[nb] = t

            HWQ = 2 + NKT * 128  # bias cols + p0 tiles
            HX = XBW // 2
            wk_sb = wpool.tile([128, WQW], BF16, tag="wk", name="wk")
            nc.sync.dma_start(out=wk_sb[:, :HWQ], in_=wk_d[:, :HWQ])
            wq_sb = wpool.tile([128, WQW], BF16, tag="wq", name="wq")
            nc.scalar.dma_start(out=wq_sb[:, :HWQ], in_=wq_d[:, :HWQ])
            xk0 = xkp.tile([128, XBW], BF16, tag="xk", name="xk0")
            nc.sync.dma_start(out=xk0[:, :HX], in_=xk_d[:, :HX])
            xk_t[0] = xk0
            xq0 = xqp.tile([128, XBW], BF16, tag="xq", name="xq0")
            nc.scalar.dma_start(out=xq0[:, :HX], in_=xq_d[:, :HX])
            xq_t[0] = xq0
            nc.sync.dma_start(out=wk_sb[:, HWQ:], in_=wk_d[:, HWQ:])
            nc.scalar.dma_start(out=wq_sb[:, HWQ:], in_=wq_d[:, HWQ:])
            nc.sync.dma_start(out=xk0[:, HX:], in_=xk_d[:, HX:XBW])
            nc.scalar.dma_start(out=xq0[:, HX:], in_=xq_d[:, HX:XBW])

            # k-blocks gate the lead-in exp stream: split each remaining xk
            # block across BOTH rings (first half on sync, second on scalar
            # behind xq0) so xk1..xk3 land ~6us earlier each.
            def dma_xk_both(nb):
                t = xkp.tile([128, XBW], BF16, tag="xk", name=f"xk{nb}")
                nc.sync.dma_start(out=t[:, :HX], in_=xk_d[:, XBW * nb : XBW * nb + HX])
                nc.scalar.dma_start(
                    out=t[:, HX:], in_=xk_d[:, XBW * nb + HX : XBW * (nb + 1)]
                )
                xk_t[nb] = t

            dma_xk_both(1)
            dma_xk_both(2)
            dma_xk_both(3)
            dma_x(xv_t, xvp, xv_d, 0, "xv", split=4)
            dma_x(xq_t, xqp, xq_d, 1, "xq", eng=nc.scalar)
            wv_sb = wpool.tile([128, NKT * DCORE], BF16, tag="wv", name="wv")
            nc.scalar.dma_start(out=wv_sb, in_=wv_d[:, :])
            dma_x(xv_t, xvp, xv_d, 1, "xv", split=4)
            dma_x(xv_t, xvp, xv_d, 2, "xv", split=2, eng=nc.scalar)

            # bias cols -> f32 (tensor_scalar needs an f32 scalar operand)
            bq_sb = persist.tile([128, 2], F32, tag="bq", name="bq")
            nc.vector.tensor_copy(out=bq_sb, in_=wq_sb[:, 0:2])
            bk_sb = persist.tile([128, 2], F32, tag="bk", name="bk")
            nc.vector.tensor_copy(out=bk_sb, in_=wk_sb[:, 0:2])

            # ---- resident activations ----
            # normal layout: head-pair p, head a on rows 64a..64a+63
            # dup layout (kTd/qTd): row halves swapped (head a on the other
            # half) so the i=1 scores matmul can sit on the opposite K rows
            qT_sb = [persist.tile([128, S], BF16, tag=f"qT{p}", name=f"qT{p}") for p in range(2)]
            kT_sb = [persist.tile([128, S], BF16, tag=f"kT{p}", name=f"kT{p}") for p in range(2)]
            qTd_sb = [persist.tile([128, S], BF16, tag=f"qTd{p}", name=f"qTd{p}") for p in range(2)]
            kTd_sb = [persist.tile([128, S], BF16, tag=f"kTd{p}", name=f"kTd{p}") for p in range(2)]
            v_aug = persist.tile([128, NMT * DCORE], BF16, tag="vaug", name="vaug")
            otn_sb = [persist.tile([128, S], BF16, tag=f"otn{p}", name=f"otn{p}") for p in range(2)]

            def proj_block(which, p, nb):
                """qT/kT (+ row-swapped dup) for pair p, 512-token block nb."""
                w_sb, x_t, dst, dstd, b_sb = (
                    (wq_sb, xq_t, qT_sb, qTd_sb, bq_sb)
                    if which == "q"
                    else (wk_sb, xk_t, kT_sb, kTd_sb, bk_sb)
                )
                tb = slice(QB * nb, QB * (nb + 1))
                ps = mix.tile([128, QB], F32, tag="mix", name="psproj")
                for kt in range(NKT):
                    c0 = 2 + 1024 * p + 128 * kt
                    nc.tensor.matmul(
                        ps,
                        w_sb[:, c0 : c0 + 128],
                        x_t[nb][:, QB * kt : QB * (kt + 1)],
                        start=(kt == 0),
                        stop=(kt == NKT - 1),
                    )
                nc.vector.tensor_scalar_add(dst[p][:, tb], ps, b_sb[:, p : p + 1])
                # row-swapped duplicate, split DVE/GpSimd so the pair runs in
                # ~0.7us (it gates the i=1 scores matmuls of this block)
                nc.vector.tensor_copy(
                    out=dstd[p][64:128, tb], in_=dst[p][0:64, tb]
                )
                nc.gpsimd.tensor_copy(
                    out=dstd[p][0:64, tb], in_=dst[p][64:128, tb]
                )

            def v_tile(m):
                """one 128-token tile of v. xv is packed m-major (1024
                contiguous cols per tile) so tile m unblocks as soon as its
                quarter of the xv block lands."""
                nb, c0 = m // 4, (m % 4) * 1024
                ps_v = mix.tile([128, QB], F32, tag="mix", name="psv")
                for kt in range(NKT):
                    nc.tensor.matmul(
                        ps_v[:, :DCORE],
                        xv_t[nb][:, c0 + 128 * kt : c0 + 128 * (kt + 1)],
                        wv_sb[:, DCORE * kt : DCORE * (kt + 1)],
                        start=(kt == 0),
                        stop=(kt == NKT - 1),
                    )
                nc.vector.tensor_copy(
                    out=v_aug[:, DCORE * m : DCORE * (m + 1)],
                    in_=ps_v[:, :DCORE],
                )

            se_ring = {}  # (u, j, a) -> tile

            def scores_half(u, j, a):
                """scores+exp for unit u, kv pair (2j, 2j+1), head a.
                The two kv tiles run as a concurrent pair on opposite K=64
                row halves (i=1 via the row-swapped dup layout); both write
                halves of R[a] and depend only on exp(a) of the prior step."""
                p, qb = divmod(u, NQB)
                qs = slice(QB * qb, QB * (qb + 1))
                reg = pss.tile([128, 1024], F32, tag=f"R{a}", name=f"R{a}")
                kv0, kv1 = 2 * j, 2 * j + 1
                r0 = slice(64 * a, 64 * a + 64)
                r1 = slice(64 * (1 - a), 64 * (1 - a) + 64)
                nc.tensor.matmul(
                    reg[:, 0:512],
                    kT_sb[p][r0, 128 * kv0 : 128 * (kv0 + 1)],
                    qT_sb[p][r0, qs],
                    start=True, stop=True,
                    tile_position=(64 * a, 0),
                )
                nc.tensor.matmul(
                    reg[:, 512:1024],
                    kTd_sb[p][r1, 128 * kv1 : 128 * (kv1 + 1)],
                    qTd_sb[p][r1, qs],
                    start=True, stop=True,
                    tile_position=(64 * (1 - a), 0),
                )
                if with_mask:
                    for i, kv in ((0, kv0), (1, kv1)):
                        mt = mpool.tile([128, QB], F32, tag="mask", name="maskt")
                        nc.sync.dma_start(
                            out=mt, in_=maskT[128 * kv : 128 * (kv + 1), qs]
                        )
                        nc.vector.tensor_add(
                            reg[:, 512 * i : 512 * (i + 1)],
                            reg[:, 512 * i : 512 * (i + 1)],
                            mt,
                        )
                t = sexp.tile([128, 1024], BF16, tag=f"se{a}", name=f"se{a}")
                se_ring[(u, j, a)] = t
                nc.scalar.activation(
                    out=t, in_=reg, func=mybir.ActivationFunctionType.Exp
                )

            # AV PSUM banks per unit:
            #   psA: rows 0-63 AV head a (pos 0); row 64 d_a(i0); row 96 d_a(i1)
            #   psB: rows 64-127 AV head b (pos 64); row 0 d_b(i0); row 32 d_b(i1)
            av_ps = {}

            def av_chunk(u, j, on_mix=False):
                """AV pair slots + denominator quad for kv pair (2j, 2j+1)."""
                p, qb = divmod(u, NQB)
                if j == 0:
                    pool_, tagA, tagB = (
                        (mix, "mix", "mix") if on_mix else (pso, "psoA", "psoB")
                    )
                    av_ps[(u, 0)] = pool_.tile([128, QB], F32, tag=tagA, name="psoA")
                    av_ps[(u, 1)] = pool_.tile([128, QB], F32, tag=tagB, name="psoB")
                psA = av_ps[(u, 0)]
                psB = av_ps[(u, 1)]
                se_a = se_ring[(u, j, 0)]
                se_b = se_ring[(u, j, 1)]
                first = j == 0
                last = j == NJ - 1
                for i in range(2):
                    kv = 2 * j + i
                    sl = slice(512 * i, 512 * (i + 1))
                    h0 = DCORE * kv + 64 * (2 * p)
                    nc.tensor.matmul(
                        psA[0:64, :], v_aug[:, h0 : h0 + 64], se_a[:, sl],
                        start=(first and i == 0), stop=(last and i == 1),
                        tile_position=(0, 0), skip_group_check=True,
                    )
                    nc.tensor.matmul(
                        psB[64:128, :], v_aug[:, h0 + 64 : h0 + 128], se_b[:, sl],
                        start=(first and i == 0), stop=(last and i == 1),
                        tile_position=(0, 64), skip_group_check=True,
                    )
                nc.tensor.matmul(
                    psA[64:65, :], ones_col, se_a[:, 0:512],
                    start=first, stop=last,
                    tile_position=(0, 64), skip_group_check=True,
                )
                nc.tensor.matmul(
                    psA[96:97, :], ones_col, se_a[:, 512:1024],
                    start=first, stop=last,
                    tile_position=(0, 96), skip_group_check=True,
                )
                nc.tensor.matmul(
                    psB[0:1, :], ones_col, se_b[:, 0:512],
                    start=first, stop=last,
                    tile_position=(0, 0), skip_group_check=True,
                )
                nc.tensor.matmul(
                    psB[32:33, :], ones_col, se_b[:, 512:1024],
                    start=first, stop=last,
                    tile_position=(0, 32), skip_group_check=True,
                )

            def av_norm(u):
                """normalize unit u's AV accumulators into otn. One full-bank
                copy per head releases the PSUM accumulator immediately; the
                divide chain then runs off the SBUF scratch."""
                p, qb = divmod(u, NQB)
                qs = slice(QB * qb, QB * (qb + 1))
                psA = av_ps.pop((u, 0))
                psB = av_ps.pop((u, 1))
                scr = {}
                for a, ps_o in ((0, psA), (1, psB)):
                    scr[a] = small.tile([128, QB], F32, tag="scr", name="scr")
                    nc.vector.tensor_copy(out=scr[a], in_=ps_o)
                for a, ps_o, avsl, d0, d1 in (
                    (0, psA, slice(0, 64), 64, 96),
                    (1, psB, slice(64, 128), 0, 32),
                ):
                    s = scr[a]
                    zrow = small.tile([1, QB], F32, tag="zrow", name="zrow")
                    nc.vector.tensor_add(
                        zrow, s[d0 : d0 + 1, :], ps_o[d1 : d1 + 1, :]
                    )
                    rc = small.tile([1, QB], F32, tag="rc", name="rc")
                    nc.vector.reciprocal_approx_fast(out=rc, in_=zrow[:, :])
                    bc = small.tile([128, QB], F32, tag="bc", name="bc")
                    nc.gpsimd.partition_broadcast(bc, rc[:, :])
                    nc.vector.tensor_mul(
                        otn_sb[p][64 * a : 64 * (a + 1), qs],
                        s[avsl, :],
                        bc[avsl, :],
                    )

            def post_mtile(m, tail_idx=None):
                """post projection + output DMA for one 128-token tile. In
                the tail (after the last exp) the scores PSUM banks are free:
                borrow an R tile per m-tile so the cast/psum-recycle ladder
                has 4 half-slots in flight instead of mix's 2."""
                ms = slice(128 * m, 128 * (m + 1))
                o_t = outs.tile([128, D], BF16, tag="outp", name="outp")
                reg = None
                if tail_idx is not None:
                    reg = pss.tile(
                        [128, 1024], F32, tag=f"R{tail_idx % 2}", name="psptail"
                    )
                for nj in range(2):
                    if reg is not None:
                        ps_p = reg[:, 512 * nj : 512 * (nj + 1)]
                    else:
                        ps_p = mix.tile([128, 512], F32, tag="mix", name="psp")
                    for kp in range(2):
                        nc.tensor.matmul(
                            ps_p,
                            otn_sb[kp][:, ms],
                            wp_box["wp"][:, D * kp + 512 * nj : D * kp + 512 * (nj + 1)],
                            start=(kp == 0),
                            stop=(kp == 1),
                            skip_group_check=True,
                        )
                    # tail casts: the first tail block's casts go to the
                    # (idle, post-exp) ACT engine — the DVE FIFO still holds
                    # the final norm chains and casts queued behind them
                    # would stall the PSUM recycle. Later blocks run after
                    # the norms drain, so alternate ACT/DVE to double the
                    # cast bandwidth.
                    if tail_idx is not None and (tail_idx < 8 or nj == 0):
                        nc.scalar.copy(
                            out=o_t[:, 512 * nj : 512 * (nj + 1)], in_=ps_p
                        )
                    else:
                        nc.vector.tensor_copy(
                            out=o_t[:, 512 * nj : 512 * (nj + 1)], in_=ps_p
                        )
                nc.sync.dma_start(out=out_d[ms, :], in_=o_t)

            def post_block(qb, tail=False):
                for mi in range(QB // 128):
                    m = (QB * qb) // 128 + mi
                    post_mtile(m, tail_idx=(m if tail else None))

            # ================= emission schedule =================
            # lead-in: unit 0 scores while inputs stream in. The scores/exp
            # chain is high-priority so the static scheduler starts the exp
            # stream as soon as each k-block's projection lands, instead of
            # batching projections first.
            proj_block("k", 0, 0)
            proj_block("q", 0, 0)
            with tc.high_priority():
                scores_half(0, 0, 0)
                scores_half(0, 0, 1)
                scores_half(0, 1, 0)
                scores_half(0, 1, 1)
            proj_block("k", 0, 1)
            proj_block("k", 1, 0)
            with tc.high_priority():
                scores_half(0, 2, 0)
                scores_half(0, 2, 1)
            with tc.high_priority():
                scores_half(0, 3, 0)
                scores_half(0, 3, 1)
            proj_block("k", 1, 1)
            proj_block("k", 0, 2)
            with tc.high_priority():
                scores_half(0, 4, 0)
                scores_half(0, 4, 1)
            with tc.high_priority():
                scores_half(0, 5, 0)
                scores_half(0, 5, 1)
            proj_block("q", 0, 1)
            proj_block("k", 0, 3)
            with tc.high_priority():
                scores_half(0, 6, 0)
                scores_half(0, 6, 1)
            with tc.high_priority():
                scores_half(0, 7, 0)
                scores_half(0, 7, 1)
            proj_block("k", 1, 2)
            proj_block("q", 1, 0)

            wp_box = {}

            def self_wp():
                t = wpool.tile([128, 2 * D], BF16, tag="wp", name="wp")
                nc.sync.dma_start(out=t, in_=wp_d[:, :])
                wp_box["wp"] = t

            # period 0: scores U1; fillers ordered DMA-independent first.
            # q02/q03 must land here (U2/U3 scores read them in periods 1/2)
            # and v0-7 must land here (U0's AV consumes all 16 tiles in
            # period 1); m-major xv packing keeps the v tiles from gating.
            def fillers_p0():
                yield lambda: proj_block("k", 1, 3)
                yield lambda: dma_x(xq_t, xqp, xq_d, 2, "xq")
                yield lambda: dma_x(xq_t, xqp, xq_d, 3, "xq")
                yield lambda: self_wp()
                yield lambda: proj_block("q", 1, 1)
                for m in range(0, 4):
                    yield (lambda m=m: v_tile(m))
                yield lambda: dma_x(xv_t, xvp, xv_d, 3, "xv")
                for m in range(4, 8):
                    yield (lambda m=m: v_tile(m))
                yield lambda: proj_block("q", 0, 2)
                yield lambda: proj_block("q", 1, 2)

            fl = list(fillers_p0())
            fi = 0
            for j in range(NJ):
                scores_half(4, j, 0)
                scores_half(4, j, 1)
                take = (len(fl) * (j + 1)) // NJ
                while fi < take:
                    fl[fi]()
                    fi += 1

            # periods 1..6: scores U(t+1), AV U(t) (U0 lag-1 at t=1; U1's
            # catch-up runs interleaved ON MIX during period 2 so the exp
            # stream never starves); per-j emission interleaves the AV/d
            # slots between the two scores halves so neither engine stalls.
            # AV chunks for units >= 2 run one j-step early ("pattern B":
            # chunks 0,1 at step 1, chunk j+1 at step j, norm at step 7) so
            # each unit's norm completes ~one step before the next unit's
            # first AV chunk needs the PSUM banks back. Period 6 additionally
            # runs U7's AV (on mix) one j-step behind its exps; posts for
            # q-blocks 0 ride period 5's slack and 1-3 drain in the tail.
            SCORES_SEQ = {1: 1, 2: 5, 3: 2, 4: 6, 5: 3, 6: 7}
            AV_PSO = {1: 0, 2: 4, 3: 5, 4: 2, 5: 6, 6: 3}
            AV_MIX = {3: 1}
            extras = {
                1: [(lambda m=m: v_tile(m)) for m in range(8, 16)]
                + [lambda: proj_block("q", 1, 3)],
                4: [lambda: proj_block("q", 0, 3)]
                + [(lambda m=m: post_mtile(m)) for m in range(0, 4)],
                5: [(lambda m=m: post_mtile(m)) for m in range(4, 8)],
            }
            for t in range(1, 7):
                us = SCORES_SEQ[t]
                ua = AV_PSO[t]
                um = AV_MIX.get(t)
                shifted = t >= 2
                ext = extras.get(t, [])
                ei = 0
                take = (len(ext) * 2) // NJ
                while ei < take:
                    ext[ei]()
                    ei += 1
                for j in range(NJ):
                    take = min(len(ext), (len(ext) * (j + 3)) // NJ)
                    while ei < take:
                        ext[ei]()
                        ei += 1
                    scores_half(us, j, 0)
                    if not shifted:
                        av_chunk(ua, j)
                    elif j == 1:
                        av_chunk(ua, 0)
                        av_chunk(ua, 1)
                        av_chunk(ua, 2)
                    elif 2 <= j <= NJ - 2:
                        av_chunk(ua, j + 1)
                    elif j == NJ - 1:
                        av_norm(ua)
                    scores_half(us, j, 1)
                    if um is not None:
                        av_chunk(um, j, on_mix=True)
                    if t == 6 and j >= 1:
                        av_chunk(7, j - 1, on_mix=True)
                if not shifted:
                    av_norm(ua)
                if um is not None:
                    av_norm(um)
                if t == 6:
                    av_chunk(7, NJ - 1, on_mix=True)
                    av_norm(7)

            # tail: post(2) is ungated at tail start (norms 2 and 6 are
            # done); post(3) waits norm(3)/norm(7) which run concurrently on
            # the DVE. Both use the freed scores PSUM banks.
            post_block(2, tail=True)
            post_block(3, tail=True)

    nc.compile()
    return nc


def _get_program(with_mask: bool):
    if with_mask not in _CACHE:
        _CACHE[with_mask] = _build(with_mask)
    return _CACHE[with_mask]


def _pack_rows(arr, bf16):
    """[8*128, F] -> [128, 8*F] tile-major (kt-major in free dim)."""
    kt, f = arr.shape[0] // 128, arr.shape[1]
    return np.ascontiguousarray(
        arr.reshape(kt, 128, f).transpose(1, 0, 2).reshape(128, kt * f)
    ).astype(bf16)


def _pack_w_page(wT_s, bias, bf16):
    """[128, 2 + 2*1024] wq/wk page: 2 leading bias columns (column p =
    bias for pair p's 128 dims), then p-major kt tiles."""
    page = np.zeros((128, WQW), np.float32)
    page[:, 0:2] = bias.reshape(2, 128).T
    for p in range(2):
        for kt in range(NKT):
            blk = wT_s[128 * kt : 128 * (kt + 1), 128 * p : 128 * (p + 1)]
            page[:, 2 + 1024 * p + 128 * kt : 2 + 1024 * p + 128 * (kt + 1)] = blk
    return np.ascontiguousarray(page).astype(bf16)


def _pack_x(x, bf16):
    """x [S, D] -> packed [128, NQB*XBW]: block nb, then kt, then token."""
    xT = x.T.astype(np.float32)  # [D, S]
    a = xT.reshape(NKT, 128, NQB, QB).transpose(1, 2, 0, 3)  # [128, nb, kt, c]
    return np.ascontiguousarray(a.reshape(128, NQB * XBW)).astype(bf16)


def _pack_xv(x, bf16):
    """x [S, D] -> [128, NQB*XBW] m-major: block nb, then 128-token tile
    within the block, then kt, then token — so v_tile(m) depends only on
    its own 1024-column quarter of the block DMA."""
    xT = x.T.astype(np.float32)  # [D, S]
    a = xT.reshape(NKT, 128, NQB, 4, 128).transpose(1, 2, 3, 0, 4)
    return np.ascontiguousarray(a.reshape(128, NQB * XBW)).astype(bf16)


def _prepare(query, key, value, mask, Wq, bq, Wk, bk, Wv, bv, Wpost, bpost,
             per_dim_scale):
    f32 = np.float32
    query = np.asarray(query, f32)
    key = np.asarray(key, f32)
    value = np.asarray(value, f32)
    mask = np.asarray(mask, f32)
    Wq = np.asarray(Wq, f32)
    bq = np.asarray(bq, f32)
    Wk = np.asarray(Wk, f32)
    bk = np.asarray(bk, f32)
    Wv = np.asarray(Wv, f32)
    bv = np.asarray(bv, f32)
    Wpost = np.asarray(Wpost, f32)
    bpost = np.asarray(bpost, f32)
    per_dim_scale = np.asarray(per_dim_scale, f32)

    r_softplus_0 = 1.442695041
    scale = (r_softplus_0 / np.sqrt(DK)) * np.log1p(np.exp(per_dim_scale))
    scale = scale.astype(f32)  # [DK]
    scale_tiled = np.tile(scale, HPC)  # [DCORE]

    with_mask = bool(np.any(mask))
    nc = _get_program(with_mask)

    bf16 = ml_dtypes.bfloat16
    in_maps = []
    for c in range(8):
        b = c // 4
        g = c % 4
        dsl = slice(DCORE * g, DCORE * (g + 1))

        wqT_s = Wq[dsl, :].T * scale_tiled[None, :]  # [D, 256] f32
        wkT_s = Wk[dsl, :].T
        wvT_s = Wv[dsl, :].T  # [D, 256]
        wpT_s = Wpost[:, dsl].T  # [256, 1024]

        m = {
            "xq": _pack_x(query[b], bf16),
            "xk": _pack_x(key[b], bf16),
            "xv": _pack_xv(value[b], bf16),
            "wq": _pack_w_page(wqT_s, bq[dsl] * scale_tiled, bf16),
            "wk": _pack_w_page(wkT_s, bk[dsl], bf16),
            "wv": _pack_rows(wvT_s, bf16),
            "wp": _pack_rows(wpT_s, bf16),
        }
        if with_mask:
            m["maskT"] = np.ascontiguousarray(mask[0, 0].T)
        in_maps.append(m)

    return nc, in_maps, bpost


def kernel(query, key, value, mask, Wq, bq, Wk, bk, Wv, bv, Wpost, bpost,
           per_dim_scale):
    global LAST_RESULTS
    nc, in_maps, bpost = _prepare(
        query, key, value, mask, Wq, bq, Wk, bk, Wv, bv, Wpost, bpost,
        per_dim_scale,
    )
    trace = os.environ.get("BASS_TRACE", "") not in ("", "0")
    if trace:
        _ensure_ntff_hook()
    res = run_bass_kernel_spmd(nc, in_maps, list(range(8)), trace=trace)
    LAST_RESULTS = res

    out = np.zeros((B, S, D), np.float32)
    for c in range(8):
        out[c // 4] += np.asarray(res.results[c]["out_p"], np.float32)
    # softmax rows sum to 1, so the value-projection bias contributes the
    # constant vector bv @ Wpost^T to every output row (folded here).
    bias = np.asarray(bpost, np.float32) + np.asarray(bv, np.float32) @ np.asarray(
        Wpost, np.float32
    ).T
    out += bias[None, None, :]
    return out


# revision 47
# speedup vs baseline: 1.0146x; 1.0146x over previous
"""Multi-head attention (B=2, S=2048, D=1024, H=16) on 8 trn2 cores.

Sharding: core c handles batch b = c//4 and heads 4g..4g+3 where g = c%4
(tensor-parallel on heads: Wq/Wk/Wv column-sharded, Wpost row-sharded).
Each core emits a partial [S, D] output; host sums the 4 partials per batch
and adds bpost.

v3b pipeline: ACT-paced stream with concurrent tiled matmuls. HW-verified
(microbenchmark + production traces): matmuls whose 32-row/col array strips
are disjoint run concurrently (4ns stagger) when adjacent in the stream.
Per (unit, j) step the PE does ~5 slot-times instead of 8:
  - scores: 2 slots. Pair = (kv tile 2j, kv tile 2j+1) of the SAME head on
    opposite K=64 row halves; the i=1 member reads a row-swapped duplicate
    of kT/qT (built by GpSimd copies) so both members share one exp/PSUM
    dependency and the scheduler keeps them adjacent.
  - AV: 2 slots (both heads via M=64 col halves, no ones row).
  - softmax denominators: 1 slot (quad of M=1 ones-contractions into spare
    PSUM rows of the AV accumulator banks).
The exp stream on ACT (2 x 1107ns per step) is the pacer; projections, v
tiles and post ride the PE slack. Post outputs DMA straight from PSUM.
PE is pre-warmed with dummy matmuls so the HAM clock gate opens before
real data lands; biases ride as 2 leading columns of the wq/wk pages.
"""

import os

import numpy as np
import ml_dtypes

import concourse.bass as bass
import concourse.tile as tile
from concourse import bacc
from concourse import mybir
from concourse.bass_utils import run_bass_kernel_spmd

F32 = mybir.dt.float32
BF16 = mybir.dt.bfloat16

B, S, D, H = 2, 2048, 1024, 16
DK = D // H          # 64
HPC = 4              # heads per core
DCORE = HPC * DK     # 256 output dims per core
NKT = D // 128       # 8 contraction tiles over d_in
NMT = S // 128       # 16 token tiles
QB = 512             # query block
NQB = S // QB        # 4
NKV = S // 128       # 16 kv tiles
NJ = NKV // 2        # 8 kv-pair chunks per unit
XBW = NKT * QB       # 4096 packed x columns per 512-token block
WQW = 2 + 2 * NKT * 128   # wq/wk page: 2 bias cols + p-major kt tiles

_CACHE = {}
LAST_RESULTS = None


def _ensure_ntff_hook():
    """The agent image's antenv lacks axon_hooks; synthesize it and register
    the ctypes NTFF profiling hook so trace=True yields exec times."""
    import sys
    import types

    try:
        from antenv import axon_hooks  # noqa: F401
        return
    except ImportError:
        pass
    mod = types.ModuleType("antenv.axon_hooks")
    _state = {"hook": None}
    mod.set_axon_ntff_profile_hook = lambda h: _state.__setitem__("hook", h)
    mod.get_axon_ntff_profile_hook = lambda: _state["hook"]
    sys.modules["antenv.axon_hooks"] = mod
    import antenv

    antenv.axon_hooks = mod
    try:
        import trn_agent_boot.trn_boot as _tb

        hook = _tb._ntff_profile_via_ctypes("/opt/axon/libaxon_pjrt.so")
        mod.set_axon_ntff_profile_hook(hook)
    except Exception:
        pass


def _build(with_mask: bool):
    nc = bacc.Bacc(None, target_bir_lowering=False)

    xq_d = nc.declare_dram_parameter("xq", [128, NQB * XBW], BF16, isOutput=False)
    xk_d = nc.declare_dram_parameter("xk", [128, NQB * XBW], BF16, isOutput=False)
    xv_d = nc.declare_dram_parameter("xv", [128, NQB * XBW], BF16, isOutput=False)
    wq_d = nc.declare_dram_parameter("wq", [128, WQW], BF16, isOutput=False)
    wk_d = nc.declare_dram_parameter("wk", [128, WQW], BF16, isOutput=False)
    wv_d = nc.declare_dram_parameter("wv", [128, NKT * DCORE], BF16, isOutput=False)
    wp_d = nc.declare_dram_parameter("wp", [128, 2 * D], BF16, isOutput=False)
    maskT = None
    if with_mask:
        maskT = nc.declare_dram_parameter("maskT", [S, S], F32, isOutput=False)
    out_d = nc.declare_dram_parameter("out_p", [S, D], BF16, isOutput=True)

    with tile.TileContext(nc) as tc:
        with (
            tc.tile_pool(name="persist", bufs=1) as persist,
            tc.tile_pool(name="wpool", bufs=1) as wpool,
            tc.tile_pool(name="xkp", bufs=2) as xkp,
            tc.tile_pool(name="xqp", bufs=2) as xqp,
            tc.tile_pool(name="xvp", bufs=3) as xvp,
            tc.tile_pool(name="sexp", bufs=16) as sexp,
            tc.tile_pool(name="small", bufs=2) as small,
            tc.tile_pool(name="outs", bufs=2) as outs,
            tc.tile_pool(name="mpool", bufs=2) as mpool,
            tc.tile_pool(name="pss", bufs=1, space="PSUM") as pss,
            tc.tile_pool(name="pso", bufs=1, space="PSUM") as pso,
            tc.tile_pool(name="mix", bufs=2, space="PSUM") as mix,
        ):
            # ---- PE pre-warm (HAM clock gate) + ACT table warm ----
            warm_sb = persist.tile([128, 512], BF16, tag="warm_sb", name="warm_sb")
            nc.vector.memset(warm_sb, 0.01)
            ones_col = persist.tile([128, 1], BF16, tag="ones", name="ones")
            nc.vector.memset(ones_col, 1.0)
            warm_act = small.tile([1, 128], F32, tag="warm", name="warm")
            nc.scalar.activation(
                out=warm_act, in_=warm_sb[0:1, 0:128],
                func=mybir.ActivationFunctionType.Exp,
            )
            warm_ps = mix.tile([128, 512], F32, tag="mix", name="warmps")
            for _ in range(22):
                nc.tensor.matmul(
                    warm_ps, warm_sb[:, 0:128], warm_sb[:, :],
                    start=True, stop=True, skip_group_check=True,
                )

            # ---- input DMAs in deadline order ----
            xk_t = {}
            xq_t = {}
            xv_t = {}

            def dma_x(store, pool, src, nb, tag, split=1, eng=None):
                t = pool.tile([128, XBW], BF16, tag=tag, name=f"{tag}{nb}")
                hw = XBW // split
                for h in range(split):
                    (eng or nc.sync).dma_start(
                        out=t[:, hw * h : hw * (h + 1)],
                        in_=src[:, XBW * nb + hw * h : XBW * nb + hw * (h + 1)],
                    )
                store[nb] = t

            HWQ = 2 + NKT * 128  # bias cols + p0 tiles
            HX = XBW // 2
            wk_sb = wpool.tile([128, WQW], BF16, tag="wk", name="wk")
            nc.sync.dma_start(out=wk_sb[:, :HWQ], in_=wk_d[:, :HWQ])
            wq_sb = wpool.tile([128, WQW], BF16, tag="wq", name="wq")
            nc.scalar.dma_start(out=wq_sb[:, :HWQ], in_=wq_d[:, :HWQ])
            xk0 = xkp.tile([128, XBW], BF16, tag="xk", name="xk0")
            nc.sync.dma_start(out=xk0[:, :HX], in_=xk_d[:, :HX])
            xk_t[0] = xk0
            xq0 = xqp.tile([128, XBW], BF16, tag="xq", name="xq0")
            nc.scalar.dma_start(out=xq0[:, :HX], in_=xq_d[:, :HX])
            xq_t[0] = xq0
            nc.sync.dma_start(out=wk_sb[:, HWQ:], in_=wk_d[:, HWQ:])
            nc.scalar.dma_start(out=wq_sb[:, HWQ:], in_=wq_d[:, HWQ:])
            nc.sync.dma_start(out=xk0[:, HX:], in_=xk_d[:, HX:XBW])
            nc.scalar.dma_start(out=xq0[:, HX:], in_=xq_d[:, HX:XBW])

            # k-blocks gate the lead-in exp stream: split each remaining xk
            # block across BOTH rings (first half on sync, second on scalar
            # behind xq0) so xk1..xk3 land ~6us earlier each.
            def dma_xk_both(nb):
                t = xkp.tile([128, XBW], BF16, tag="xk", name=f"xk{nb}")
                nc.sync.dma_start(out=t[:, :HX], in_=xk_d[:, XBW * nb : XBW * nb + HX])
                nc.scalar.dma_start(
                    out=t[:, HX:], in_=xk_d[:, XBW * nb + HX : XBW * (nb + 1)]
                )
                xk_t[nb] = t

            dma_xk_both(1)
            dma_xk_both(2)
            dma_xk_both(3)
            dma_x(xv_t, xvp, xv_d, 0, "xv", split=4)
            dma_x(xq_t, xqp, xq_d, 1, "xq", eng=nc.scalar)
            wv_sb = wpool.tile([128, NKT * DCORE], BF16, tag="wv", name="wv")
            nc.scalar.dma_start(out=wv_sb, in_=wv_d[:, :])
            dma_x(xv_t, xvp, xv_d, 1, "xv", split=4)
            dma_x(xv_t, xvp, xv_d, 2, "xv", split=2, eng=nc.scalar)

            # bias cols -> f32 (tensor_scalar needs an f32 scalar operand)
            bq_sb = persist.tile([128, 2], F32, tag="bq", name="bq")
            nc.vector.tensor_copy(out=bq_sb, in_=wq_sb[:, 0:2])
            bk_sb = persist.tile([128, 2], F32, tag="bk", name="bk")
            nc.vector.tensor_copy(out=bk_sb, in_=wk_sb[:, 0:2])

            # ---- resident activations ----
            # normal layout: head-pair p, head a on rows 64a..64a+63
            # dup layout (kTd/qTd): row halves swapped (head a on the other
            # half) so the i=1 scores matmul can sit on the opposite K rows
            qT_sb = [persist.tile([128, S], BF16, tag=f"qT{p}", name=f"qT{p}") for p in range(2)]
            kT_sb = [persist.tile([128, S], BF16, tag=f"kT{p}", name=f"kT{p}") for p in range(2)]
            qTd_sb = [persist.tile([128, S], BF16, tag=f"qTd{p}", name=f"qTd{p}") for p in range(2)]
            kTd_sb = [persist.tile([128, S], BF16, tag=f"kTd{p}", name=f"kTd{p}") for p in range(2)]
            v_aug = persist.tile([128, NMT * DCORE], BF16, tag="vaug", name="vaug")
            otn_sb = [persist.tile([128, S], BF16, tag=f"otn{p}", name=f"otn{p}") for p in range(2)]

            def proj_block(which, p, nb):
                """qT/kT (+ row-swapped dup) for pair p, 512-token block nb."""
                w_sb, x_t, dst, dstd, b_sb = (
                    (wq_sb, xq_t, qT_sb, qTd_sb, bq_sb)
                    if which == "q"
                    else (wk_sb, xk_t, kT_sb, kTd_sb, bk_sb)
                )
                tb = slice(QB * nb, QB * (nb + 1))
                ps = mix.tile([128, QB], F32, tag="mix", name="psproj")
                for kt in range(NKT):
                    c0 = 2 + 1024 * p + 128 * kt
                    nc.tensor.matmul(
                        ps,
                        w_sb[:, c0 : c0 + 128],
                        x_t[nb][:, QB * kt : QB * (kt + 1)],
                        start=(kt == 0),
                        stop=(kt == NKT - 1),
                    )
                nc.vector.tensor_scalar_add(dst[p][:, tb], ps, b_sb[:, p : p + 1])
                # row-swapped duplicate, split DVE/GpSimd so the pair runs in
                # ~0.7us (it gates the i=1 scores matmuls of this block)
                nc.vector.tensor_copy(
                    out=dstd[p][64:128, tb], in_=dst[p][0:64, tb]
                )
                nc.gpsimd.tensor_copy(
                    out=dstd[p][0:64, tb], in_=dst[p][64:128, tb]
                )

            def v_tile(m):
                """one 128-token tile of v. xv is packed m-major (1024
                contiguous cols per tile) so tile m unblocks as soon as its
                quarter of the xv block lands."""
                nb, c0 = m // 4, (m % 4) * 1024
                ps_v = mix.tile([128, QB], F32, tag="mix", name="psv")
                for kt in range(NKT):
                    nc.tensor.matmul(
                        ps_v[:, :DCORE],
                        xv_t[nb][:, c0 + 128 * kt : c0 + 128 * (kt + 1)],
                        wv_sb[:, DCORE * kt : DCORE * (kt + 1)],
                        start=(kt == 0),
                        stop=(kt == NKT - 1),
                    )
                nc.vector.tensor_copy(
                    out=v_aug[:, DCORE * m : DCORE * (m + 1)],
                    in_=ps_v[:, :DCORE],
                )

            se_ring = {}  # (u, j, a) -> tile

            def scores_half(u, j, a):
                """scores+exp for unit u, kv pair (2j, 2j+1), head a.
                The two kv tiles run as a concurrent pair on opposite K=64
                row halves (i=1 via the row-swapped dup layout); both write
                halves of R[a] and depend only on exp(a) of the prior step."""
                p, qb = divmod(u, NQB)
                qs = slice(QB * qb, QB * (qb + 1))
                reg = pss.tile([128, 1024], F32, tag=f"R{a}", name=f"R{a}")
                kv0, kv1 = 2 * j, 2 * j + 1
                r0 = slice(64 * a, 64 * a + 64)
                r1 = slice(64 * (1 - a), 64 * (1 - a) + 64)
                nc.tensor.matmul(
                    reg[:, 0:512],
                    kT_sb[p][r0, 128 * kv0 : 128 * (kv0 + 1)],
                    qT_sb[p][r0, qs],
                    start=True, stop=True,
                    tile_position=(64 * a, 0),
                )
                nc.tensor.matmul(
                    reg[:, 512:1024],
                    kTd_sb[p][r1, 128 * kv1 : 128 * (kv1 + 1)],
                    qTd_sb[p][r1, qs],
                    start=True, stop=True,
                    tile_position=(64 * (1 - a), 0),
                )
                if with_mask:
                    for i, kv in ((0, kv0), (1, kv1)):
                        mt = mpool.tile([128, QB], F32, tag="mask", name="maskt")
                        nc.sync.dma_start(
                            out=mt, in_=maskT[128 * kv : 128 * (kv + 1), qs]
                        )
                        nc.vector.tensor_add(
                            reg[:, 512 * i : 512 * (i + 1)],
                            reg[:, 512 * i : 512 * (i + 1)],
                            mt,
                        )
                t = sexp.tile([128, 1024], BF16, tag=f"se{a}", name=f"se{a}")
                se_ring[(u, j, a)] = t
                nc.scalar.activation(
                    out=t, in_=reg, func=mybir.ActivationFunctionType.Exp
                )

            # AV PSUM banks per unit:
            #   psA: rows 0-63 AV head a (pos 0); row 64 d_a(i0); row 96 d_a(i1)
            #   psB: rows 64-127 AV head b (pos 64); row 0 d_b(i0); row 32 d_b(i1)
            av_ps = {}

            def av_chunk(u, j, on_mix=False):
                """AV pair slots + denominator quad for kv pair (2j, 2j+1)."""
                p, qb = divmod(u, NQB)
                if j == 0:
                    pool_, tagA, tagB = (
                        (mix, "mix", "mix") if on_mix else (pso, "psoA", "psoB")
                    )
                    av_ps[(u, 0)] = pool_.tile([128, QB], F32, tag=tagA, name="psoA")
                    av_ps[(u, 1)] = pool_.tile([128, QB], F32, tag=tagB, name="psoB")
                psA = av_ps[(u, 0)]
                psB = av_ps[(u, 1)]
                se_a = se_ring[(u, j, 0)]
                se_b = se_ring[(u, j, 1)]
                first = j == 0
                last = j == NJ - 1
                for i in range(2):
                    kv = 2 * j + i
                    sl = slice(512 * i, 512 * (i + 1))
                    h0 = DCORE * kv + 64 * (2 * p)
                    nc.tensor.matmul(
                        psA[0:64, :], v_aug[:, h0 : h0 + 64], se_a[:, sl],
                        start=(first and i == 0), stop=(last and i == 1),
                        tile_position=(0, 0), skip_group_check=True,
                    )
                    nc.tensor.matmul(
                        psB[64:128, :], v_aug[:, h0 + 64 : h0 + 128], se_b[:, sl],
                        start=(first and i == 0), stop=(last and i == 1),
                        tile_position=(0, 64), skip_group_check=True,
                    )
                nc.tensor.matmul(
                    psA[64:65, :], ones_col, se_a[:, 0:512],
                    start=first, stop=last,
                    tile_position=(0, 64), skip_group_check=True,
                )
                nc.tensor.matmul(
                    psA[96:97, :], ones_col, se_a[:, 512:1024],
                    start=first, stop=last,
                    tile_position=(0, 96), skip_group_check=True,
                )
                nc.tensor.matmul(
                    psB[0:1, :], ones_col, se_b[:, 0:512],
                    start=first, stop=last,
                    tile_position=(0, 0), skip_group_check=True,
                )
                nc.tensor.matmul(
                    psB[32:33, :], ones_col, se_b[:, 512:1024],
                    start=first, stop=last,
                    tile_position=(0, 32), skip_group_check=True,
                )

            def av_norm(u):
                """normalize unit u's AV accumulators into otn. One full-bank
                copy per head releases the PSUM accumulator immediately; the
                divide chain then runs off the SBUF scratch."""
                p, qb = divmod(u, NQB)
                qs = slice(QB * qb, QB * (qb + 1))
                psA = av_ps.pop((u, 0))
                psB = av_ps.pop((u, 1))
                scr = {}
                for a, ps_o in ((0, psA), (1, psB)):
                    scr[a] = small.tile([128, QB], F32, tag="scr", name="scr")
                    nc.vector.tensor_copy(out=scr[a], in_=ps_o)
                for a, ps_o, avsl, d0, d1 in (
                    (0, psA, slice(0, 64), 64, 96),
                    (1, psB, slice(64, 128), 0, 32),
                ):
                    s = scr[a]
                    zrow = small.tile([1, QB], F32, tag="zrow", name="zrow")
                    nc.vector.tensor_add(
                        zrow, s[d0 : d0 + 1, :], ps_o[d1 : d1 + 1, :]
                    )
                    rc = small.tile([1, QB], F32, tag="rc", name="rc")
                    nc.vector.reciprocal_approx_fast(out=rc, in_=zrow[:, :])
                    bc = small.tile([128, QB], F32, tag="bc", name="bc")
                    nc.gpsimd.partition_broadcast(bc, rc[:, :])
                    nc.vector.tensor_mul(
                        otn_sb[p][64 * a : 64 * (a + 1), qs],
                        s[avsl, :],
                        bc[avsl, :],
                    )

            def post_mtile(m, tail_idx=None):
                """post projection + output DMA for one 128-token tile. In
                the tail (after the last exp) the scores PSUM banks are free:
                borrow an R tile per m-tile so the cast/psum-recycle ladder
                has 4 half-slots in flight instead of mix's 2."""
                ms = slice(128 * m, 128 * (m + 1))
                o_t = outs.tile([128, D], BF16, tag="outp", name="outp")
                reg = None
                if tail_idx is not None:
                    reg = pss.tile(
                        [128, 1024], F32, tag=f"R{tail_idx % 2}", name="psptail"
                    )
                for nj in range(2):
                    if reg is not None:
                        ps_p = reg[:, 512 * nj : 512 * (nj + 1)]
                    else:
                        ps_p = mix.tile([128, 512], F32, tag="mix", name="psp")
                    for kp in range(2):
                        nc.tensor.matmul(
                            ps_p,
                            otn_sb[kp][:, ms],
                            wp_box["wp"][:, D * kp + 512 * nj : D * kp + 512 * (nj + 1)],
                            start=(kp == 0),
                            stop=(kp == 1),
                            skip_group_check=True,
                        )
                    # tail casts: the first tail block's casts go to the
                    # (idle, post-exp) ACT engine — the DVE FIFO still holds
                    # the final norm chains and casts queued behind them
                    # would stall the PSUM recycle. Later blocks run after
                    # the norms drain, so alternate ACT/DVE to double the
                    # cast bandwidth.
                    if tail_idx is not None and (tail_idx < 8 or nj == 0):
                        nc.scalar.copy(
                            out=o_t[:, 512 * nj : 512 * (nj + 1)], in_=ps_p
                        )
                    else:
                        nc.vector.tensor_copy(
                            out=o_t[:, 512 * nj : 512 * (nj + 1)], in_=ps_p
                        )
                nc.sync.dma_start(out=out_d[ms, :], in_=o_t)

            def post_block(qb, tail=False):
                for mi in range(QB // 128):
                    m = (QB * qb) // 128 + mi
                    post_mtile(m, tail_idx=(m if tail else None))

            # ================= emission schedule =================
            # lead-in: unit 0 scores while inputs stream in. The scores/exp
            # chain is high-priority so the static scheduler starts the exp
            # stream as soon as each k-block's projection lands, instead of
            # batching projections first.
            proj_block("k", 0, 0)
            proj_block("q", 0, 0)
            with tc.high_priority():
                scores_half(0, 0, 0)
                scores_half(0, 0, 1)
                scores_half(0, 1, 0)
                scores_half(0, 1, 1)
            proj_block("k", 0, 1)
            proj_block("k", 1, 0)
            with tc.high_priority():
                scores_half(0, 2, 0)
                scores_half(0, 2, 1)
            with tc.high_priority():
                scores_half(0, 3, 0)
                scores_half(0, 3, 1)
            proj_block("k", 1, 1)
            proj_block("k", 0, 2)
            with tc.high_priority():
                scores_half(0, 4, 0)
                scores_half(0, 4, 1)
            with tc.high_priority():
                scores_half(0, 5, 0)
                scores_half(0, 5, 1)
            proj_block("q", 0, 1)
            proj_block("k", 0, 3)
            with tc.high_priority():
                scores_half(0, 6, 0)
                scores_half(0, 6, 1)
            with tc.high_priority():
                scores_half(0, 7, 0)
                scores_half(0, 7, 1)
            proj_block("k", 1, 2)

            wp_box = {}

            def self_wp():
                t = wpool.tile([128, 2 * D], BF16, tag="wp", name="wp")
                nc.sync.dma_start(out=t, in_=wp_d[:, :])
                wp_box["wp"] = t

            # period 0: scores U1; fillers ordered DMA-independent first.
            # q02/q03 must land here (U2/U3 scores read them in periods 1/2)
            # and v0-7 must land here (U0's AV consumes all 16 tiles in
            # period 1); m-major xv packing keeps the v tiles from gating.
            def fillers_p0():
                yield lambda: proj_block("q", 1, 0)
                yield lambda: dma_x(xq_t, xqp, xq_d, 2, "xq")
                yield lambda: proj_block("q", 1, 1)
                yield lambda: dma_x(xq_t, xqp, xq_d, 3, "xq")
                yield lambda: self_wp()
                yield lambda: proj_block("k", 1, 3)
                for m in range(0, 4):
                    yield (lambda m=m: v_tile(m))
                yield lambda: dma_x(xv_t, xvp, xv_d, 3, "xv")
                for m in range(4, 8):
                    yield (lambda m=m: v_tile(m))
                yield lambda: proj_block("q", 0, 2)
                yield lambda: proj_block("q", 0, 3)

            fl = list(fillers_p0())
            fi = 0
            for j in range(NJ):
                scores_half(1, j, 0)
                scores_half(1, j, 1)
                take = (len(fl) * (j + 1)) // NJ
                while fi < take:
                    fl[fi]()
                    fi += 1

            # periods 1..6: scores U(t+1), AV U(t) (U0 lag-1 at t=1; U1's
            # catch-up runs interleaved ON MIX during period 2 so the exp
            # stream never starves); per-j emission interleaves the AV/d
            # slots between the two scores halves so neither engine stalls.
            # AV chunks for units >= 2 run one j-step early ("pattern B":
            # chunks 0,1 at step 1, chunk j+1 at step j, norm at step 7) so
            # each unit's norm completes ~one step before the next unit's
            # first AV chunk needs the PSUM banks back. Period 6 additionally
            # runs U7's AV (on mix) one j-step behind its exps; posts for
            # q-blocks 0 ride period 5's slack and 1-3 drain in the tail.
            extras = {
                1: [(lambda m=m: v_tile(m)) for m in range(8, 16)],
                4: [lambda: proj_block("q", 1, 2)],
                5: [lambda: proj_block("q", 1, 3)]
                + [(lambda m=m: post_mtile(m)) for m in range(0, 4)],
            }
            for t in range(1, 7):
                us = t + 1
                ua = 0 if t == 1 else t
                shifted = t >= 2
                ext = extras.get(t, [])
                ei = 0
                take = (len(ext) * 2) // NJ
                while ei < take:
                    ext[ei]()
                    ei += 1
                for j in range(NJ):
                    take = min(len(ext), (len(ext) * (j + 3)) // NJ)
                    while ei < take:
                        ext[ei]()
                        ei += 1
                    scores_half(us, j, 0)
                    if not shifted:
                        av_chunk(ua, j)
                    elif j == 1:
                        av_chunk(ua, 0)
                        av_chunk(ua, 1)
                        av_chunk(ua, 2)
                    elif 2 <= j <= NJ - 2:
                        av_chunk(ua, j + 1)
                    elif j == NJ - 1:
                        av_norm(ua)
                    scores_half(us, j, 1)
                    if t == 2:
                        av_chunk(1, j, on_mix=True)
                    if t == 3 and j == 3:
                        # U1's deferred norm: its mix accumulators are only
                        # needed again at t=4, and running the DVE chain here
                        # keeps it clear of the t2/t3 boundary
                        av_norm(1)
                    if t == 6 and j >= 1:
                        av_chunk(7, j - 1, on_mix=True)
                if not shifted:
                    av_norm(ua)
                if t == 6:
                    av_chunk(7, NJ - 1, on_mix=True)
                    av_norm(7)

            # tail: remaining post blocks (their second otn halves come from
            # units 5, 6 and 7), on the freed scores PSUM banks.
            post_block(1, tail=True)
            post_block(2, tail=True)
            post_block(3, tail=True)

    nc.compile()
    return nc


def _get_program(with_mask: bool):
    if with_mask not in _CACHE:
        _CACHE[with_mask] = _build(with_mask)
    return _CACHE[with_mask]


def _pack_rows(arr, bf16):
    """[8*128, F] -> [128, 8*F] tile-major (kt-major in free dim)."""
    kt, f = arr.shape[0] // 128, arr.shape[1]
    return np.ascontiguousarray(
        arr.reshape(kt, 128, f).transpose(1, 0, 2).reshape(128, kt * f)
    ).astype(bf16)


def _pack_w_page(wT_s, bias, bf16):
    """[128, 2 + 2*1024] wq/wk page: 2 leading bias columns (column p =
    bias for pair p's 128 dims), then p-major kt tiles."""
    page = np.zeros((128, WQW), np.float32)
    page[:, 0:2] = bias.reshape(2, 128).T
    for p in range(2):
        for kt in range(NKT):
            blk = wT_s[128 * kt : 128 * (kt + 1), 128 * p : 128 * (p + 1)]
            page[:, 2 + 1024 * p + 128 * kt : 2 + 1024 * p + 128 * (kt + 1)] = blk
    return np.ascontiguousarray(page).astype(bf16)


def _pack_x(x, bf16):
    """x [S, D] -> packed [128, NQB*XBW]: block nb, then kt, then token."""
    xT = x.T.astype(np.float32)  # [D, S]
    a = xT.reshape(NKT, 128, NQB, QB).transpose(1, 2, 0, 3)  # [128, nb, kt, c]
    return np.ascontiguousarray(a.reshape(128, NQB * XBW)).astype(bf16)


def _pack_xv(x, bf16):
    """x [S, D] -> [128, NQB*XBW] m-major: block nb, then 128-token tile
    within the block, then kt, then token — so v_tile(m) depends only on
    its own 1024-column quarter of the block DMA."""
    xT = x.T.astype(np.float32)  # [D, S]
    a = xT.reshape(NKT, 128, NQB, 4, 128).transpose(1, 2, 3, 0, 4)
    return np.ascontiguousarray(a.reshape(128, NQB * XBW)).astype(bf16)


def _prepare(query, key, value, mask, Wq, bq, Wk, bk, Wv, bv, Wpost, bpost,
             per_dim_scale):
    f32 = np.float32
    query = np.asarray(query, f32)
    key = np.asarray(key, f32)
    value = np.asarray(value, f32)
    mask = np.asarray(mask, f32)
    Wq = np.asarray(Wq, f32)
    bq = np.asarray(bq, f32)
    Wk = np.asarray(Wk, f32)
    bk = np.asarray(bk, f32)
    Wv = np.asarray(Wv, f32)
    bv = np.asarray(bv, f32)
    Wpost = np.asarray(Wpost, f32)
    bpost = np.asarray(bpost, f32)
    per_dim_scale = np.asarray(per_dim_scale, f32)

    r_softplus_0 = 1.442695041
    scale = (r_softplus_0 / np.sqrt(DK)) * np.log1p(np.exp(per_dim_scale))
    scale = scale.astype(f32)  # [DK]
    scale_tiled = np.tile(scale, HPC)  # [DCORE]

    with_mask = bool(np.any(mask))
    nc = _get_program(with_mask)

    bf16 = ml_dtypes.bfloat16
    in_maps = []
    for c in range(8):
        b = c // 4
        g = c % 4
        dsl = slice(DCORE * g, DCORE * (g + 1))

        wqT_s = Wq[dsl, :].T * scale_tiled[None, :]  # [D, 256] f32
        wkT_s = Wk[dsl, :].T
        wvT_s = Wv[dsl, :].T  # [D, 256]
        wpT_s = Wpost[:, dsl].T  # [256, 1024]

        m = {
            "xq": _pack_x(query[b], bf16),
            "xk": _pack_x(key[b], bf16),
            "xv": _pack_xv(value[b], bf16),
            "wq": _pack_w_page(wqT_s, bq[dsl] * scale_tiled, bf16),
            "wk": _pack_w_page(wkT_s, bk[dsl], bf16),
            "wv": _pack_rows(wvT_s, bf16),
            "wp": _pack_rows(wpT_s, bf16),
        }
        if with_mask:
            m["maskT"] = np.ascontiguousarray(mask[0, 0].T)
        in_maps.append(m)

    return nc, in_maps, bpost


def kernel(query, key, value, mask, Wq, bq, Wk, bk, Wv, bv, Wpost, bpost,
           per_dim_scale):
    global LAST_RESULTS
    nc, in_maps, bpost = _prepare(
        query, key, value, mask, Wq, bq, Wk, bk, Wv, bv, Wpost, bpost,
        per_dim_scale,
    )
    trace = os.environ.get("BASS_TRACE", "") not in ("", "0")
    if trace:
        _ensure_ntff_hook()
    res = run_bass_kernel_spmd(nc, in_maps, list(range(8)), trace=trace)
    LAST_RESULTS = res

    out = np.zeros((B, S, D), np.float32)
    for c in range(8):
        out[c // 4] += np.asarray(res.results[c]["out_p"], np.float32)
    # softmax rows sum to 1, so the value-projection bias contributes the
    # constant vector bv @ Wpost^T to every output row (folded here).
    bias = np.asarray(bpost, np.float32) + np.asarray(bv, np.float32) @ np.asarray(
        Wpost, np.float32
    ).T
    out += bias[None, None, :]
    return out


# revision 48
# speedup vs baseline: 1.0193x; 1.0047x over previous
"""Multi-head attention (B=2, S=2048, D=1024, H=16) on 8 trn2 cores.

Sharding: core c handles batch b = c//4 and heads 4g..4g+3 where g = c%4
(tensor-parallel on heads: Wq/Wk/Wv column-sharded, Wpost row-sharded).
Each core emits a partial [S, D] output; host sums the 4 partials per batch
and adds bpost.

v3b pipeline: ACT-paced stream with concurrent tiled matmuls. HW-verified
(microbenchmark + production traces): matmuls whose 32-row/col array strips
are disjoint run concurrently (4ns stagger) when adjacent in the stream.
Per (unit, j) step the PE does ~5 slot-times instead of 8:
  - scores: 2 slots. Pair = (kv tile 2j, kv tile 2j+1) of the SAME head on
    opposite K=64 row halves; the i=1 member reads a row-swapped duplicate
    of kT/qT (built by GpSimd copies) so both members share one exp/PSUM
    dependency and the scheduler keeps them adjacent.
  - AV: 2 slots (both heads via M=64 col halves, no ones row).
  - softmax denominators: 1 slot (quad of M=1 ones-contractions into spare
    PSUM rows of the AV accumulator banks).
The exp stream on ACT (2 x 1107ns per step) is the pacer; projections, v
tiles and post ride the PE slack. Post outputs DMA straight from PSUM.
PE is pre-warmed with dummy matmuls so the HAM clock gate opens before
real data lands; biases ride as 2 leading columns of the wq/wk pages.
"""

import os

import numpy as np
import ml_dtypes

import concourse.bass as bass
import concourse.tile as tile
from concourse import bacc
from concourse import mybir
from concourse.bass_utils import run_bass_kernel_spmd

F32 = mybir.dt.float32
BF16 = mybir.dt.bfloat16

B, S, D, H = 2, 2048, 1024, 16
DK = D // H          # 64
HPC = 4              # heads per core
DCORE = HPC * DK     # 256 output dims per core
NKT = D // 128       # 8 contraction tiles over d_in
NMT = S // 128       # 16 token tiles
QB = 512             # query block
NQB = S // QB        # 4
NKV = S // 128       # 16 kv tiles
NJ = NKV // 2        # 8 kv-pair chunks per unit
XBW = NKT * QB       # 4096 packed x columns per 512-token block
WQW = 2 + 2 * NKT * 128   # wq/wk page: 2 bias cols + p-major kt tiles

_CACHE = {}
LAST_RESULTS = None


def _ensure_ntff_hook():
    """The agent image's antenv lacks axon_hooks; synthesize it and register
    the ctypes NTFF profiling hook so trace=True yields exec times."""
    import sys
    import types

    try:
        from antenv import axon_hooks  # noqa: F401
        return
    except ImportError:
        pass
    mod = types.ModuleType("antenv.axon_hooks")
    _state = {"hook": None}
    mod.set_axon_ntff_profile_hook = lambda h: _state.__setitem__("hook", h)
    mod.get_axon_ntff_profile_hook = lambda: _state["hook"]
    sys.modules["antenv.axon_hooks"] = mod
    import antenv

    antenv.axon_hooks = mod
    try:
        import trn_agent_boot.trn_boot as _tb

        hook = _tb._ntff_profile_via_ctypes("/opt/axon/libaxon_pjrt.so")
        mod.set_axon_ntff_profile_hook(hook)
    except Exception:
        pass


def _build(with_mask: bool):
    nc = bacc.Bacc(None, target_bir_lowering=False)

    xq_d = nc.declare_dram_parameter("xq", [128, NQB * XBW], BF16, isOutput=False)
    xk_d = nc.declare_dram_parameter("xk", [128, NQB * XBW], BF16, isOutput=False)
    xv_d = nc.declare_dram_parameter("xv", [128, NQB * XBW], BF16, isOutput=False)
    wq_d = nc.declare_dram_parameter("wq", [128, WQW], BF16, isOutput=False)
    wk_d = nc.declare_dram_parameter("wk", [128, WQW], BF16, isOutput=False)
    wv_d = nc.declare_dram_parameter("wv", [128, NKT * DCORE], BF16, isOutput=False)
    wp_d = nc.declare_dram_parameter("wp", [128, 2 * D], BF16, isOutput=False)
    maskT = None
    if with_mask:
        maskT = nc.declare_dram_parameter("maskT", [S, S], F32, isOutput=False)
    out_d = nc.declare_dram_parameter("out_p", [S, D], BF16, isOutput=True)

    with tile.TileContext(nc) as tc:
        with (
            tc.tile_pool(name="persist", bufs=1) as persist,
            tc.tile_pool(name="wpool", bufs=1) as wpool,
            tc.tile_pool(name="xkp", bufs=2) as xkp,
            tc.tile_pool(name="xqp", bufs=2) as xqp,
            tc.tile_pool(name="xvp", bufs=3) as xvp,
            tc.tile_pool(name="sexp", bufs=16) as sexp,
            tc.tile_pool(name="small", bufs=2) as small,
            tc.tile_pool(name="outs", bufs=2) as outs,
            tc.tile_pool(name="mpool", bufs=2) as mpool,
            tc.tile_pool(name="pss", bufs=1, space="PSUM") as pss,
            tc.tile_pool(name="pso", bufs=1, space="PSUM") as pso,
            tc.tile_pool(name="mix", bufs=2, space="PSUM") as mix,
        ):
            # ---- PE pre-warm (HAM clock gate) + ACT table warm ----
            warm_sb = persist.tile([128, 512], BF16, tag="warm_sb", name="warm_sb")
            nc.vector.memset(warm_sb, 0.01)
            ones_col = persist.tile([128, 1], BF16, tag="ones", name="ones")
            nc.vector.memset(ones_col, 1.0)
            warm_act = small.tile([1, 128], F32, tag="warm", name="warm")
            nc.scalar.activation(
                out=warm_act, in_=warm_sb[0:1, 0:128],
                func=mybir.ActivationFunctionType.Exp,
            )
            warm_ps = mix.tile([128, 512], F32, tag="mix", name="warmps")
            for _ in range(22):
                nc.tensor.matmul(
                    warm_ps, warm_sb[:, 0:128], warm_sb[:, :],
                    start=True, stop=True, skip_group_check=True,
                )

            # ---- input DMAs in deadline order ----
            xk_t = {}
            xq_t = {}
            xv_t = {}

            def dma_x(store, pool, src, nb, tag, split=1, eng=None):
                t = pool.tile([128, XBW], BF16, tag=tag, name=f"{tag}{nb}")
                hw = XBW // split
                for h in range(split):
                    (eng or nc.sync).dma_start(
                        out=t[:, hw * h : hw * (h + 1)],
                        in_=src[:, XBW * nb + hw * h : XBW * nb + hw * (h + 1)],
                    )
                store[nb] = t

            HWQ = 2 + NKT * 128  # bias cols + p0 tiles
            HX = XBW // 2
            wk_sb = wpool.tile([128, WQW], BF16, tag="wk", name="wk")
            nc.sync.dma_start(out=wk_sb[:, :HWQ], in_=wk_d[:, :HWQ])
            wq_sb = wpool.tile([128, WQW], BF16, tag="wq", name="wq")
            nc.scalar.dma_start(out=wq_sb[:, :HWQ], in_=wq_d[:, :HWQ])
            xk0 = xkp.tile([128, XBW], BF16, tag="xk", name="xk0")
            nc.sync.dma_start(out=xk0[:, :HX], in_=xk_d[:, :HX])
            xk_t[0] = xk0
            xq0 = xqp.tile([128, XBW], BF16, tag="xq", name="xq0")
            nc.scalar.dma_start(out=xq0[:, :HX], in_=xq_d[:, :HX])
            xq_t[0] = xq0
            nc.sync.dma_start(out=wk_sb[:, HWQ:], in_=wk_d[:, HWQ:])
            nc.scalar.dma_start(out=wq_sb[:, HWQ:], in_=wq_d[:, HWQ:])
            nc.sync.dma_start(out=xk0[:, HX:], in_=xk_d[:, HX:XBW])
            nc.scalar.dma_start(out=xq0[:, HX:], in_=xq_d[:, HX:XBW])

            # k-blocks gate the lead-in exp stream: split each remaining xk
            # block across BOTH rings (first half on sync, second on scalar
            # behind xq0) so xk1..xk3 land ~6us earlier each.
            def dma_xk_both(nb):
                t = xkp.tile([128, XBW], BF16, tag="xk", name=f"xk{nb}")
                nc.sync.dma_start(out=t[:, :HX], in_=xk_d[:, XBW * nb : XBW * nb + HX])
                nc.scalar.dma_start(
                    out=t[:, HX:], in_=xk_d[:, XBW * nb + HX : XBW * (nb + 1)]
                )
                xk_t[nb] = t

            dma_xk_both(1)
            dma_xk_both(2)
            dma_xk_both(3)
            dma_x(xv_t, xvp, xv_d, 0, "xv", split=4)
            dma_x(xq_t, xqp, xq_d, 1, "xq", eng=nc.scalar)
            wv_sb = wpool.tile([128, NKT * DCORE], BF16, tag="wv", name="wv")
            nc.scalar.dma_start(out=wv_sb, in_=wv_d[:, :])
            dma_x(xv_t, xvp, xv_d, 1, "xv", split=4)
            dma_x(xv_t, xvp, xv_d, 2, "xv", split=2, eng=nc.scalar)

            # bias cols -> f32 (tensor_scalar needs an f32 scalar operand)
            bq_sb = persist.tile([128, 2], F32, tag="bq", name="bq")
            nc.vector.tensor_copy(out=bq_sb, in_=wq_sb[:, 0:2])
            bk_sb = persist.tile([128, 2], F32, tag="bk", name="bk")
            nc.vector.tensor_copy(out=bk_sb, in_=wk_sb[:, 0:2])

            # ---- resident activations ----
            # normal layout: head-pair p, head a on rows 64a..64a+63
            # dup layout (kTd/qTd): row halves swapped (head a on the other
            # half) so the i=1 scores matmul can sit on the opposite K rows
            qT_sb = [persist.tile([128, S], BF16, tag=f"qT{p}", name=f"qT{p}") for p in range(2)]
            kT_sb = [persist.tile([128, S], BF16, tag=f"kT{p}", name=f"kT{p}") for p in range(2)]
            qTd_sb = [persist.tile([128, S], BF16, tag=f"qTd{p}", name=f"qTd{p}") for p in range(2)]
            kTd_sb = [persist.tile([128, S], BF16, tag=f"kTd{p}", name=f"kTd{p}") for p in range(2)]
            v_aug = persist.tile([128, NMT * DCORE], BF16, tag="vaug", name="vaug")
            otn_sb = [persist.tile([128, S], BF16, tag=f"otn{p}", name=f"otn{p}") for p in range(2)]

            def proj_block(which, p, nb):
                """qT/kT (+ row-swapped dup) for pair p, 512-token block nb."""
                w_sb, x_t, dst, dstd, b_sb = (
                    (wq_sb, xq_t, qT_sb, qTd_sb, bq_sb)
                    if which == "q"
                    else (wk_sb, xk_t, kT_sb, kTd_sb, bk_sb)
                )
                tb = slice(QB * nb, QB * (nb + 1))
                ps = mix.tile([128, QB], F32, tag="mix", name="psproj")
                for kt in range(NKT):
                    c0 = 2 + 1024 * p + 128 * kt
                    nc.tensor.matmul(
                        ps,
                        w_sb[:, c0 : c0 + 128],
                        x_t[nb][:, QB * kt : QB * (kt + 1)],
                        start=(kt == 0),
                        stop=(kt == NKT - 1),
                    )
                nc.vector.tensor_scalar_add(dst[p][:, tb], ps, b_sb[:, p : p + 1])
                # row-swapped duplicate, split DVE/GpSimd so the pair runs in
                # ~0.7us (it gates the i=1 scores matmuls of this block)
                nc.vector.tensor_copy(
                    out=dstd[p][64:128, tb], in_=dst[p][0:64, tb]
                )
                nc.gpsimd.tensor_copy(
                    out=dstd[p][0:64, tb], in_=dst[p][64:128, tb]
                )

            def v_tile(m):
                """one 128-token tile of v. xv is packed m-major (1024
                contiguous cols per tile) so tile m unblocks as soon as its
                quarter of the xv block lands."""
                nb, c0 = m // 4, (m % 4) * 1024
                ps_v = mix.tile([128, QB], F32, tag="mix", name="psv")
                for kt in range(NKT):
                    nc.tensor.matmul(
                        ps_v[:, :DCORE],
                        xv_t[nb][:, c0 + 128 * kt : c0 + 128 * (kt + 1)],
                        wv_sb[:, DCORE * kt : DCORE * (kt + 1)],
                        start=(kt == 0),
                        stop=(kt == NKT - 1),
                    )
                nc.vector.tensor_copy(
                    out=v_aug[:, DCORE * m : DCORE * (m + 1)],
                    in_=ps_v[:, :DCORE],
                )

            se_ring = {}  # (u, j, a) -> tile

            def scores_half(u, j, a):
                """scores+exp for unit u, kv pair (2j, 2j+1), head a.
                The two kv tiles run as a concurrent pair on opposite K=64
                row halves (i=1 via the row-swapped dup layout); both write
                halves of R[a] and depend only on exp(a) of the prior step."""
                p, qb = divmod(u, NQB)
                qs = slice(QB * qb, QB * (qb + 1))
                reg = pss.tile([128, 1024], F32, tag=f"R{a}", name=f"R{a}")
                kv0, kv1 = 2 * j, 2 * j + 1
                r0 = slice(64 * a, 64 * a + 64)
                r1 = slice(64 * (1 - a), 64 * (1 - a) + 64)
                nc.tensor.matmul(
                    reg[:, 0:512],
                    kT_sb[p][r0, 128 * kv0 : 128 * (kv0 + 1)],
                    qT_sb[p][r0, qs],
                    start=True, stop=True,
                    tile_position=(64 * a, 0),
                )
                nc.tensor.matmul(
                    reg[:, 512:1024],
                    kTd_sb[p][r1, 128 * kv1 : 128 * (kv1 + 1)],
                    qTd_sb[p][r1, qs],
                    start=True, stop=True,
                    tile_position=(64 * (1 - a), 0),
                )
                if with_mask:
                    for i, kv in ((0, kv0), (1, kv1)):
                        mt = mpool.tile([128, QB], F32, tag="mask", name="maskt")
                        nc.sync.dma_start(
                            out=mt, in_=maskT[128 * kv : 128 * (kv + 1), qs]
                        )
                        nc.vector.tensor_add(
                            reg[:, 512 * i : 512 * (i + 1)],
                            reg[:, 512 * i : 512 * (i + 1)],
                            mt,
                        )
                t = sexp.tile([128, 1024], BF16, tag=f"se{a}", name=f"se{a}")
                se_ring[(u, j, a)] = t
                nc.scalar.activation(
                    out=t, in_=reg, func=mybir.ActivationFunctionType.Exp
                )

            # AV PSUM banks per unit:
            #   psA: rows 0-63 AV head a (pos 0); row 64 d_a(i0); row 96 d_a(i1)
            #   psB: rows 64-127 AV head b (pos 64); row 0 d_b(i0); row 32 d_b(i1)
            av_ps = {}

            def av_chunk(u, j, on_mix=False):
                """AV pair slots + denominator quad for kv pair (2j, 2j+1)."""
                p, qb = divmod(u, NQB)
                if j == 0:
                    pool_, tagA, tagB = (
                        (mix, "mix", "mix") if on_mix else (pso, "psoA", "psoB")
                    )
                    av_ps[(u, 0)] = pool_.tile([128, QB], F32, tag=tagA, name="psoA")
                    av_ps[(u, 1)] = pool_.tile([128, QB], F32, tag=tagB, name="psoB")
                psA = av_ps[(u, 0)]
                psB = av_ps[(u, 1)]
                se_a = se_ring[(u, j, 0)]
                se_b = se_ring[(u, j, 1)]
                first = j == 0
                last = j == NJ - 1
                for i in range(2):
                    kv = 2 * j + i
                    sl = slice(512 * i, 512 * (i + 1))
                    h0 = DCORE * kv + 64 * (2 * p)
                    nc.tensor.matmul(
                        psA[0:64, :], v_aug[:, h0 : h0 + 64], se_a[:, sl],
                        start=(first and i == 0), stop=(last and i == 1),
                        tile_position=(0, 0), skip_group_check=True,
                    )
                    nc.tensor.matmul(
                        psB[64:128, :], v_aug[:, h0 + 64 : h0 + 128], se_b[:, sl],
                        start=(first and i == 0), stop=(last and i == 1),
                        tile_position=(0, 64), skip_group_check=True,
                    )
                nc.tensor.matmul(
                    psA[64:65, :], ones_col, se_a[:, 0:512],
                    start=first, stop=last,
                    tile_position=(0, 64), skip_group_check=True,
                )
                nc.tensor.matmul(
                    psA[96:97, :], ones_col, se_a[:, 512:1024],
                    start=first, stop=last,
                    tile_position=(0, 96), skip_group_check=True,
                )
                nc.tensor.matmul(
                    psB[0:1, :], ones_col, se_b[:, 0:512],
                    start=first, stop=last,
                    tile_position=(0, 0), skip_group_check=True,
                )
                nc.tensor.matmul(
                    psB[32:33, :], ones_col, se_b[:, 512:1024],
                    start=first, stop=last,
                    tile_position=(0, 32), skip_group_check=True,
                )

            def av_norm(u):
                """normalize unit u's AV accumulators into otn. One full-bank
                copy per head releases the PSUM accumulator immediately; the
                divide chain then runs off the SBUF scratch."""
                p, qb = divmod(u, NQB)
                qs = slice(QB * qb, QB * (qb + 1))
                psA = av_ps.pop((u, 0))
                psB = av_ps.pop((u, 1))
                scr = {}
                for a, ps_o in ((0, psA), (1, psB)):
                    scr[a] = small.tile([128, QB], F32, tag="scr", name="scr")
                    nc.vector.tensor_copy(out=scr[a], in_=ps_o)
                for a, ps_o, avsl, d0, d1 in (
                    (0, psA, slice(0, 64), 64, 96),
                    (1, psB, slice(64, 128), 0, 32),
                ):
                    s = scr[a]
                    zrow = small.tile([1, QB], F32, tag="zrow", name="zrow")
                    nc.vector.tensor_add(
                        zrow, s[d0 : d0 + 1, :], ps_o[d1 : d1 + 1, :]
                    )
                    rc = small.tile([1, QB], F32, tag="rc", name="rc")
                    nc.vector.reciprocal_approx_fast(out=rc, in_=zrow[:, :])
                    bc = small.tile([128, QB], F32, tag="bc", name="bc")
                    nc.gpsimd.partition_broadcast(bc, rc[:, :])
                    nc.vector.tensor_mul(
                        otn_sb[p][64 * a : 64 * (a + 1), qs],
                        s[avsl, :],
                        bc[avsl, :],
                    )

            def post_mtile(m, tail_idx=None):
                """post projection + output DMA for one 128-token tile. In
                the tail (after the last exp) the scores PSUM banks are free:
                borrow an R tile per m-tile so the cast/psum-recycle ladder
                has 4 half-slots in flight instead of mix's 2."""
                ms = slice(128 * m, 128 * (m + 1))
                o_t = outs.tile([128, D], BF16, tag="outp", name="outp")
                reg = None
                if tail_idx is not None:
                    reg = pss.tile(
                        [128, 1024], F32, tag=f"R{tail_idx % 2}", name="psptail"
                    )
                for nj in range(2):
                    if reg is not None:
                        ps_p = reg[:, 512 * nj : 512 * (nj + 1)]
                    else:
                        ps_p = mix.tile([128, 512], F32, tag="mix", name="psp")
                    for kp in range(2):
                        nc.tensor.matmul(
                            ps_p,
                            otn_sb[kp][:, ms],
                            wp_box["wp"][:, D * kp + 512 * nj : D * kp + 512 * (nj + 1)],
                            start=(kp == 0),
                            stop=(kp == 1),
                            skip_group_check=True,
                        )
                    # tail casts: the first tail block's casts go to the
                    # (idle, post-exp) ACT engine — the DVE FIFO still holds
                    # the final norm chains and casts queued behind them
                    # would stall the PSUM recycle. Later blocks run after
                    # the norms drain, so alternate ACT/DVE to double the
                    # cast bandwidth.
                    if tail_idx is not None and (tail_idx < 12 or nj == 0):
                        nc.scalar.copy(
                            out=o_t[:, 512 * nj : 512 * (nj + 1)], in_=ps_p
                        )
                    else:
                        nc.vector.tensor_copy(
                            out=o_t[:, 512 * nj : 512 * (nj + 1)], in_=ps_p
                        )
                nc.sync.dma_start(out=out_d[ms, :], in_=o_t)

            def post_block(qb, tail=False):
                for mi in range(QB // 128):
                    m = (QB * qb) // 128 + mi
                    post_mtile(m, tail_idx=(m if tail else None))

            # ================= emission schedule =================
            # lead-in: unit 0 scores while inputs stream in. The scores/exp
            # chain is high-priority so the static scheduler starts the exp
            # stream as soon as each k-block's projection lands, instead of
            # batching projections first.
            proj_block("k", 0, 0)
            proj_block("q", 0, 0)
            with tc.high_priority():
                scores_half(0, 0, 0)
                scores_half(0, 0, 1)
                scores_half(0, 1, 0)
                scores_half(0, 1, 1)
            proj_block("k", 0, 1)
            proj_block("k", 1, 0)
            with tc.high_priority():
                scores_half(0, 2, 0)
                scores_half(0, 2, 1)
            with tc.high_priority():
                scores_half(0, 3, 0)
                scores_half(0, 3, 1)
            proj_block("k", 1, 1)
            proj_block("k", 0, 2)
            with tc.high_priority():
                scores_half(0, 4, 0)
                scores_half(0, 4, 1)
            with tc.high_priority():
                scores_half(0, 5, 0)
                scores_half(0, 5, 1)
            proj_block("q", 0, 1)
            proj_block("k", 0, 3)
            with tc.high_priority():
                scores_half(0, 6, 0)
                scores_half(0, 6, 1)
            with tc.high_priority():
                scores_half(0, 7, 0)
                scores_half(0, 7, 1)
            proj_block("k", 1, 2)

            wp_box = {}

            def self_wp():
                t = wpool.tile([128, 2 * D], BF16, tag="wp", name="wp")
                nc.sync.dma_start(out=t, in_=wp_d[:, :])
                wp_box["wp"] = t

            # period 0: scores U1; fillers ordered DMA-independent first.
            # q02/q03 must land here (U2/U3 scores read them in periods 1/2)
            # and v0-7 must land here (U0's AV consumes all 16 tiles in
            # period 1); m-major xv packing keeps the v tiles from gating.
            def fillers_p0():
                yield lambda: proj_block("q", 1, 0)
                yield lambda: dma_x(xq_t, xqp, xq_d, 2, "xq")
                yield lambda: proj_block("q", 1, 1)
                yield lambda: dma_x(xq_t, xqp, xq_d, 3, "xq")
                yield lambda: self_wp()
                yield lambda: proj_block("k", 1, 3)
                for m in range(0, 4):
                    yield (lambda m=m: v_tile(m))
                yield lambda: dma_x(xv_t, xvp, xv_d, 3, "xv")
                for m in range(4, 8):
                    yield (lambda m=m: v_tile(m))
                yield lambda: proj_block("q", 0, 2)
                yield lambda: proj_block("q", 0, 3)

            fl = list(fillers_p0())
            fi = 0
            for j in range(NJ):
                scores_half(1, j, 0)
                scores_half(1, j, 1)
                take = (len(fl) * (j + 1)) // NJ
                while fi < take:
                    fl[fi]()
                    fi += 1

            # periods 1..6: scores U(t+1), AV U(t) (U0 lag-1 at t=1; U1's
            # catch-up runs interleaved ON MIX during period 2 so the exp
            # stream never starves); per-j emission interleaves the AV/d
            # slots between the two scores halves so neither engine stalls.
            # AV chunks for units >= 2 run one j-step early ("pattern B":
            # chunks 0,1 at step 1, chunk j+1 at step j, norm at step 7) so
            # each unit's norm completes ~one step before the next unit's
            # first AV chunk needs the PSUM banks back. Period 6 additionally
            # runs U7's AV (on mix) one j-step behind its exps; posts for
            # q-blocks 0 ride period 5's slack and 1-3 drain in the tail.
            extras = {
                1: [(lambda m=m: v_tile(m)) for m in range(8, 16)],
                4: [lambda: proj_block("q", 1, 2)],
                5: [lambda: proj_block("q", 1, 3)]
                + [(lambda m=m: post_mtile(m)) for m in range(0, 4)],
            }
            for t in range(1, 7):
                us = t + 1
                ua = 0 if t == 1 else t
                shifted = t >= 2
                ext = extras.get(t, [])
                ei = 0
                take = (len(ext) * 2) // NJ
                while ei < take:
                    ext[ei]()
                    ei += 1
                for j in range(NJ):
                    take = min(len(ext), (len(ext) * (j + 3)) // NJ)
                    while ei < take:
                        ext[ei]()
                        ei += 1
                    scores_half(us, j, 0)
                    if not shifted:
                        av_chunk(ua, j)
                    elif j == 1:
                        av_chunk(ua, 0)
                        av_chunk(ua, 1)
                        av_chunk(ua, 2)
                    elif 2 <= j <= NJ - 2:
                        av_chunk(ua, j + 1)
                    elif j == NJ - 1:
                        av_norm(ua)
                    scores_half(us, j, 1)
                    if t == 2:
                        av_chunk(1, j, on_mix=True)
                    if t == 3 and j == 3:
                        # U1's deferred norm: its mix accumulators are only
                        # needed again at t=4, and running the DVE chain here
                        # keeps it clear of the t2/t3 boundary
                        av_norm(1)
                    if t == 6 and j >= 1:
                        av_chunk(7, j - 1, on_mix=True)
                if not shifted:
                    av_norm(ua)
                if t == 6:
                    av_chunk(7, NJ - 1, on_mix=True)
                    av_norm(7)

            # tail: remaining post blocks (their second otn halves come from
            # units 5, 6 and 7), on the freed scores PSUM banks.
            post_block(1, tail=True)
            post_block(2, tail=True)
            post_block(3, tail=True)

    nc.compile()
    return nc


def _get_program(with_mask: bool):
    if with_mask not in _CACHE:
        _CACHE[with_mask] = _build(with_mask)
    return _CACHE[with_mask]


def _pack_rows(arr, bf16):
    """[8*128, F] -> [128, 8*F] tile-major (kt-major in free dim)."""
    kt, f = arr.shape[0] // 128, arr.shape[1]
    return np.ascontiguousarray(
        arr.reshape(kt, 128, f).transpose(1, 0, 2).reshape(128, kt * f)
    ).astype(bf16)


def _pack_w_page(wT_s, bias, bf16):
    """[128, 2 + 2*1024] wq/wk page: 2 leading bias columns (column p =
    bias for pair p's 128 dims), then p-major kt tiles."""
    page = np.zeros((128, WQW), np.float32)
    page[:, 0:2] = bias.reshape(2, 128).T
    for p in range(2):
        for kt in range(NKT):
            blk = wT_s[128 * kt : 128 * (kt + 1), 128 * p : 128 * (p + 1)]
            page[:, 2 + 1024 * p + 128 * kt : 2 + 1024 * p + 128 * (kt + 1)] = blk
    return np.ascontiguousarray(page).astype(bf16)


def _pack_x(x, bf16):
    """x [S, D] -> packed [128, NQB*XBW]: block nb, then kt, then token."""
    xT = x.T.astype(np.float32)  # [D, S]
    a = xT.reshape(NKT, 128, NQB, QB).transpose(1, 2, 0, 3)  # [128, nb, kt, c]
    return np.ascontiguousarray(a.reshape(128, NQB * XBW)).astype(bf16)


def _pack_xv(x, bf16):
    """x [S, D] -> [128, NQB*XBW] m-major: block nb, then 128-token tile
    within the block, then kt, then token — so v_tile(m) depends only on
    its own 1024-column quarter of the block DMA."""
    xT = x.T.astype(np.float32)  # [D, S]
    a = xT.reshape(NKT, 128, NQB, 4, 128).transpose(1, 2, 3, 0, 4)
    return np.ascontiguousarray(a.reshape(128, NQB * XBW)).astype(bf16)


def _prepare(query, key, value, mask, Wq, bq, Wk, bk, Wv, bv, Wpost, bpost,
             per_dim_scale):
    f32 = np.float32
    query = np.asarray(query, f32)
    key = np.asarray(key, f32)
    value = np.asarray(value, f32)
    mask = np.asarray(mask, f32)
    Wq = np.asarray(Wq, f32)
    bq = np.asarray(bq, f32)
    Wk = np.asarray(Wk, f32)
    bk = np.asarray(bk, f32)
    Wv = np.asarray(Wv, f32)
    bv = np.asarray(bv, f32)
    Wpost = np.asarray(Wpost, f32)
    bpost = np.asarray(bpost, f32)
    per_dim_scale = np.asarray(per_dim_scale, f32)

    r_softplus_0 = 1.442695041
    scale = (r_softplus_0 / np.sqrt(DK)) * np.log1p(np.exp(per_dim_scale))
    scale = scale.astype(f32)  # [DK]
    scale_tiled = np.tile(scale, HPC)  # [DCORE]

    with_mask = bool(np.any(mask))
    nc = _get_program(with_mask)

    bf16 = ml_dtypes.bfloat16
    in_maps = []
    for c in range(8):
        b = c // 4
        g = c % 4
        dsl = slice(DCORE * g, DCORE * (g + 1))

        wqT_s = Wq[dsl, :].T * scale_tiled[None, :]  # [D, 256] f32
        wkT_s = Wk[dsl, :].T
        wvT_s = Wv[dsl, :].T  # [D, 256]
        wpT_s = Wpost[:, dsl].T  # [256, 1024]

        m = {
            "xq": _pack_x(query[b], bf16),
            "xk": _pack_x(key[b], bf16),
            "xv": _pack_xv(value[b], bf16),
            "wq": _pack_w_page(wqT_s, bq[dsl] * scale_tiled, bf16),
            "wk": _pack_w_page(wkT_s, bk[dsl], bf16),
            "wv": _pack_rows(wvT_s, bf16),
            "wp": _pack_rows(wpT_s, bf16),
        }
        if with_mask:
            m["maskT"] = np.ascontiguousarray(mask[0, 0].T)
        in_maps.append(m)

    return nc, in_maps, bpost


def kernel(query, key, value, mask, Wq, bq, Wk, bk, Wv, bv, Wpost, bpost,
           per_dim_scale):
    global LAST_RESULTS
    nc, in_maps, bpost = _prepare(
        query, key, value, mask, Wq, bq, Wk, bk, Wv, bv, Wpost, bpost,
        per_dim_scale,
    )
    trace = os.environ.get("BASS_TRACE", "") not in ("", "0")
    if trace:
        _ensure_ntff_hook()
    res = run_bass_kernel_spmd(nc, in_maps, list(range(8)), trace=trace)
    LAST_RESULTS = res

    out = np.zeros((B, S, D), np.float32)
    for c in range(8):
        out[c // 4] += np.asarray(res.results[c]["out_p"], np.float32)
    # softmax rows sum to 1, so the value-projection bias contributes the
    # constant vector bv @ Wpost^T to every output row (folded here).
    bias = np.asarray(bpost, np.float32) + np.asarray(bv, np.float32) @ np.asarray(
        Wpost, np.float32
    ).T
    out += bias[None, None, :]
    return out


# revision 49
# speedup vs baseline: 1.0198x; 1.0004x over previous
"""Multi-head attention (B=2, S=2048, D=1024, H=16) on 8 trn2 cores.

Sharding: core c handles batch b = c//4 and heads 4g..4g+3 where g = c%4
(tensor-parallel on heads: Wq/Wk/Wv column-sharded, Wpost row-sharded).
Each core emits a partial [S, D] output; host sums the 4 partials per batch
and adds bpost.

v3 pipeline: ACT-paced stream with concurrent tiled matmuls. HW-verified
(microbenchmark + production traces): matmuls whose 32-row/col array strips
are disjoint run concurrently (4ns stagger) when adjacent in the stream.
Per (unit, j) step the PE does ~5 slot-times instead of v2's 8:
  - scores: 2 slots. Pair = (kv tile 2j, kv tile 2j+1) of the SAME head on
    opposite K=64 row halves; the i=1 member reads a row-swapped duplicate
    of kT/qT (built by DVE+GpSimd copies) so both members share one
    exp/PSUM dependency and the scheduler keeps them adjacent.
  - AV: 2 slots (both heads via M=64 col halves, no ones row).
  - softmax denominators: 1 slot (quad of concurrent M=1 ones-contractions
    into spare PSUM rows of the AV accumulator banks).
The exp stream on ACT (2 x 1107ns per step) is the pacer; projections, v
tiles and post ride the PE slack. AV chunks run one j-step early so each
unit's norm chain (one full-bank PSUM copy per head, then the divide off
SBUF scratch) finishes before the next unit needs the banks. PE is
pre-warmed with dummy matmuls so the HAM clock gate opens before real
data lands; biases ride as 2 leading columns of the wq/wk pages; xk
blocks are split across both DMA rings (they gate the lead-in exp
stream, which is emitted high-priority); the tail's post blocks borrow
the freed scores PSUM banks and cast on the idle ACT engine.
"""

import os

import numpy as np
import ml_dtypes

import concourse.bass as bass
import concourse.tile as tile
from concourse import bacc
from concourse import mybir
from concourse.bass_utils import run_bass_kernel_spmd

F32 = mybir.dt.float32
BF16 = mybir.dt.bfloat16

B, S, D, H = 2, 2048, 1024, 16
DK = D // H          # 64
HPC = 4              # heads per core
DCORE = HPC * DK     # 256 output dims per core
NKT = D // 128       # 8 contraction tiles over d_in
NMT = S // 128       # 16 token tiles
QB = 512             # query block
NQB = S // QB        # 4
NKV = S // 128       # 16 kv tiles
NJ = NKV // 2        # 8 kv-pair chunks per unit
XBW = NKT * QB       # 4096 packed x columns per 512-token block
WQW = 2 + 2 * NKT * 128   # wq/wk page: 2 bias cols + p-major kt tiles

_CACHE = {}
LAST_RESULTS = None


def _ensure_ntff_hook():
    """The agent image's antenv lacks axon_hooks; synthesize it and register
    the ctypes NTFF profiling hook so trace=True yields exec times."""
    import sys
    import types

    try:
        from antenv import axon_hooks  # noqa: F401
        return
    except ImportError:
        pass
    mod = types.ModuleType("antenv.axon_hooks")
    _state = {"hook": None}
    mod.set_axon_ntff_profile_hook = lambda h: _state.__setitem__("hook", h)
    mod.get_axon_ntff_profile_hook = lambda: _state["hook"]
    sys.modules["antenv.axon_hooks"] = mod
    import antenv

    antenv.axon_hooks = mod
    try:
        import trn_agent_boot.trn_boot as _tb

        hook = _tb._ntff_profile_via_ctypes("/opt/axon/libaxon_pjrt.so")
        mod.set_axon_ntff_profile_hook(hook)
    except Exception:
        pass


def _build(with_mask: bool):
    nc = bacc.Bacc(None, target_bir_lowering=False)

    xq_d = nc.declare_dram_parameter("xq", [128, NQB * XBW], BF16, isOutput=False)
    xk_d = nc.declare_dram_parameter("xk", [128, NQB * XBW], BF16, isOutput=False)
    xv_d = nc.declare_dram_parameter("xv", [128, NQB * XBW], BF16, isOutput=False)
    wq_d = nc.declare_dram_parameter("wq", [128, WQW], BF16, isOutput=False)
    wk_d = nc.declare_dram_parameter("wk", [128, WQW], BF16, isOutput=False)
    wv_d = nc.declare_dram_parameter("wv", [128, NKT * DCORE], BF16, isOutput=False)
    wp_d = nc.declare_dram_parameter("wp", [128, 2 * D], BF16, isOutput=False)
    maskT = None
    if with_mask:
        maskT = nc.declare_dram_parameter("maskT", [S, S], F32, isOutput=False)
    out_d = nc.declare_dram_parameter("out_p", [S, D], BF16, isOutput=True)

    with tile.TileContext(nc) as tc:
        with (
            tc.tile_pool(name="persist", bufs=1) as persist,
            tc.tile_pool(name="wpool", bufs=1) as wpool,
            tc.tile_pool(name="xkp", bufs=2) as xkp,
            tc.tile_pool(name="xqp", bufs=2) as xqp,
            tc.tile_pool(name="xvp", bufs=3) as xvp,
            tc.tile_pool(name="sexp", bufs=16) as sexp,
            tc.tile_pool(name="small", bufs=2) as small,
            tc.tile_pool(name="outs", bufs=2) as outs,
            tc.tile_pool(name="mpool", bufs=2) as mpool,
            tc.tile_pool(name="pss", bufs=1, space="PSUM") as pss,
            tc.tile_pool(name="pso", bufs=1, space="PSUM") as pso,
            tc.tile_pool(name="mix", bufs=2, space="PSUM") as mix,
        ):
            # ---- PE pre-warm (HAM clock gate) + ACT table warm ----
            warm_sb = persist.tile([128, 512], BF16, tag="warm_sb", name="warm_sb")
            nc.vector.memset(warm_sb, 0.01)
            ones_col = persist.tile([128, 1], BF16, tag="ones", name="ones")
            nc.vector.memset(ones_col, 1.0)
            warm_act = small.tile([1, 128], F32, tag="warm", name="warm")
            nc.scalar.activation(
                out=warm_act, in_=warm_sb[0:1, 0:128],
                func=mybir.ActivationFunctionType.Exp,
            )
            warm_ps = mix.tile([128, 512], F32, tag="mix", name="warmps")
            for _ in range(22):
                nc.tensor.matmul(
                    warm_ps, warm_sb[:, 0:128], warm_sb[:, :],
                    start=True, stop=True, skip_group_check=True,
                )

            # ---- input DMAs in deadline order ----
            xk_t = {}
            xq_t = {}
            xv_t = {}

            def dma_x(store, pool, src, nb, tag, split=1, eng=None):
                t = pool.tile([128, XBW], BF16, tag=tag, name=f"{tag}{nb}")
                hw = XBW // split
                for h in range(split):
                    (eng or nc.sync).dma_start(
                        out=t[:, hw * h : hw * (h + 1)],
                        in_=src[:, XBW * nb + hw * h : XBW * nb + hw * (h + 1)],
                    )
                store[nb] = t

            HWQ = 2 + NKT * 128  # bias cols + p0 tiles
            HX = XBW // 2
            wk_sb = wpool.tile([128, WQW], BF16, tag="wk", name="wk")
            nc.sync.dma_start(out=wk_sb[:, :HWQ], in_=wk_d[:, :HWQ])
            wq_sb = wpool.tile([128, WQW], BF16, tag="wq", name="wq")
            nc.scalar.dma_start(out=wq_sb[:, :HWQ], in_=wq_d[:, :HWQ])
            xk0 = xkp.tile([128, XBW], BF16, tag="xk", name="xk0")
            nc.sync.dma_start(out=xk0[:, :HX], in_=xk_d[:, :HX])
            xk_t[0] = xk0
            xq0 = xqp.tile([128, XBW], BF16, tag="xq", name="xq0")
            nc.scalar.dma_start(out=xq0[:, :HX], in_=xq_d[:, :HX])
            xq_t[0] = xq0
            nc.sync.dma_start(out=wk_sb[:, HWQ:], in_=wk_d[:, HWQ:])
            nc.scalar.dma_start(out=wq_sb[:, HWQ:], in_=wq_d[:, HWQ:])
            nc.sync.dma_start(out=xk0[:, HX:], in_=xk_d[:, HX:XBW])
            nc.scalar.dma_start(out=xq0[:, HX:], in_=xq_d[:, HX:XBW])

            # k-blocks gate the lead-in exp stream: split each remaining xk
            # block across BOTH rings (first half on sync, second on scalar
            # behind xq0) so xk1..xk3 land ~6us earlier each.
            def dma_xk_both(nb):
                t = xkp.tile([128, XBW], BF16, tag="xk", name=f"xk{nb}")
                nc.sync.dma_start(out=t[:, :HX], in_=xk_d[:, XBW * nb : XBW * nb + HX])
                nc.scalar.dma_start(
                    out=t[:, HX:], in_=xk_d[:, XBW * nb + HX : XBW * (nb + 1)]
                )
                xk_t[nb] = t

            dma_xk_both(1)
            dma_xk_both(2)
            dma_xk_both(3)
            dma_x(xv_t, xvp, xv_d, 0, "xv", split=4)
            dma_x(xq_t, xqp, xq_d, 1, "xq", eng=nc.scalar)
            wv_sb = wpool.tile([128, NKT * DCORE], BF16, tag="wv", name="wv")
            nc.scalar.dma_start(out=wv_sb, in_=wv_d[:, :])
            dma_x(xv_t, xvp, xv_d, 1, "xv", split=4)
            dma_x(xv_t, xvp, xv_d, 2, "xv", split=2, eng=nc.scalar)

            # bias cols -> f32 (tensor_scalar needs an f32 scalar operand)
            bq_sb = persist.tile([128, 2], F32, tag="bq", name="bq")
            nc.vector.tensor_copy(out=bq_sb, in_=wq_sb[:, 0:2])
            bk_sb = persist.tile([128, 2], F32, tag="bk", name="bk")
            nc.vector.tensor_copy(out=bk_sb, in_=wk_sb[:, 0:2])

            # ---- resident activations ----
            # normal layout: head-pair p, head a on rows 64a..64a+63
            # dup layout (kTd/qTd): row halves swapped (head a on the other
            # half) so the i=1 scores matmul can sit on the opposite K rows
            qT_sb = [persist.tile([128, S], BF16, tag=f"qT{p}", name=f"qT{p}") for p in range(2)]
            kT_sb = [persist.tile([128, S], BF16, tag=f"kT{p}", name=f"kT{p}") for p in range(2)]
            qTd_sb = [persist.tile([128, S], BF16, tag=f"qTd{p}", name=f"qTd{p}") for p in range(2)]
            kTd_sb = [persist.tile([128, S], BF16, tag=f"kTd{p}", name=f"kTd{p}") for p in range(2)]
            v_aug = persist.tile([128, NMT * DCORE], BF16, tag="vaug", name="vaug")
            otn_sb = [persist.tile([128, S], BF16, tag=f"otn{p}", name=f"otn{p}") for p in range(2)]

            def proj_block(which, p, nb):
                """qT/kT (+ row-swapped dup) for pair p, 512-token block nb."""
                w_sb, x_t, dst, dstd, b_sb = (
                    (wq_sb, xq_t, qT_sb, qTd_sb, bq_sb)
                    if which == "q"
                    else (wk_sb, xk_t, kT_sb, kTd_sb, bk_sb)
                )
                tb = slice(QB * nb, QB * (nb + 1))
                ps = mix.tile([128, QB], F32, tag="mix", name="psproj")
                for kt in range(NKT):
                    c0 = 2 + 1024 * p + 128 * kt
                    nc.tensor.matmul(
                        ps,
                        w_sb[:, c0 : c0 + 128],
                        x_t[nb][:, QB * kt : QB * (kt + 1)],
                        start=(kt == 0),
                        stop=(kt == NKT - 1),
                    )
                nc.vector.tensor_scalar_add(dst[p][:, tb], ps, b_sb[:, p : p + 1])
                # row-swapped duplicate, split DVE/GpSimd so the pair runs in
                # ~0.7us (it gates the i=1 scores matmuls of this block)
                nc.vector.tensor_copy(
                    out=dstd[p][64:128, tb], in_=dst[p][0:64, tb]
                )
                nc.gpsimd.tensor_copy(
                    out=dstd[p][0:64, tb], in_=dst[p][64:128, tb]
                )

            def v_tile(m):
                """one 128-token tile of v. xv is packed m-major (1024
                contiguous cols per tile) so tile m unblocks as soon as its
                quarter of the xv block lands."""
                nb, c0 = m // 4, (m % 4) * 1024
                ps_v = mix.tile([128, QB], F32, tag="mix", name="psv")
                for kt in range(NKT):
                    nc.tensor.matmul(
                        ps_v[:, :DCORE],
                        xv_t[nb][:, c0 + 128 * kt : c0 + 128 * (kt + 1)],
                        wv_sb[:, DCORE * kt : DCORE * (kt + 1)],
                        start=(kt == 0),
                        stop=(kt == NKT - 1),
                    )
                nc.vector.tensor_copy(
                    out=v_aug[:, DCORE * m : DCORE * (m + 1)],
                    in_=ps_v[:, :DCORE],
                )

            se_ring = {}  # (u, j, a) -> tile

            def scores_half(u, j, a):
                """scores+exp for unit u, kv pair (2j, 2j+1), head a.
                The two kv tiles run as a concurrent pair on opposite K=64
                row halves (i=1 via the row-swapped dup layout); both write
                halves of R[a] and depend only on exp(a) of the prior step."""
                p, qb = divmod(u, NQB)
                qs = slice(QB * qb, QB * (qb + 1))
                reg = pss.tile([128, 1024], F32, tag=f"R{a}", name=f"R{a}")
                kv0, kv1 = 2 * j, 2 * j + 1
                r0 = slice(64 * a, 64 * a + 64)
                r1 = slice(64 * (1 - a), 64 * (1 - a) + 64)
                nc.tensor.matmul(
                    reg[:, 0:512],
                    kT_sb[p][r0, 128 * kv0 : 128 * (kv0 + 1)],
                    qT_sb[p][r0, qs],
                    start=True, stop=True,
                    tile_position=(64 * a, 0),
                )
                nc.tensor.matmul(
                    reg[:, 512:1024],
                    kTd_sb[p][r1, 128 * kv1 : 128 * (kv1 + 1)],
                    qTd_sb[p][r1, qs],
                    start=True, stop=True,
                    tile_position=(64 * (1 - a), 0),
                )
                if with_mask:
                    for i, kv in ((0, kv0), (1, kv1)):
                        mt = mpool.tile([128, QB], F32, tag="mask", name="maskt")
                        nc.sync.dma_start(
                            out=mt, in_=maskT[128 * kv : 128 * (kv + 1), qs]
                        )
                        nc.vector.tensor_add(
                            reg[:, 512 * i : 512 * (i + 1)],
                            reg[:, 512 * i : 512 * (i + 1)],
                            mt,
                        )
                t = sexp.tile([128, 1024], BF16, tag=f"se{a}", name=f"se{a}")
                se_ring[(u, j, a)] = t
                nc.scalar.activation(
                    out=t, in_=reg, func=mybir.ActivationFunctionType.Exp
                )

            # AV PSUM banks per unit:
            #   psA: rows 0-63 AV head a (pos 0); row 64 d_a(i0); row 96 d_a(i1)
            #   psB: rows 64-127 AV head b (pos 64); row 0 d_b(i0); row 32 d_b(i1)
            av_ps = {}

            def av_chunk(u, j, on_mix=False):
                """AV pair slots + denominator quad for kv pair (2j, 2j+1)."""
                p, qb = divmod(u, NQB)
                if j == 0:
                    pool_, tagA, tagB = (
                        (mix, "mix", "mix") if on_mix else (pso, "psoA", "psoB")
                    )
                    av_ps[(u, 0)] = pool_.tile([128, QB], F32, tag=tagA, name="psoA")
                    av_ps[(u, 1)] = pool_.tile([128, QB], F32, tag=tagB, name="psoB")
                psA = av_ps[(u, 0)]
                psB = av_ps[(u, 1)]
                se_a = se_ring[(u, j, 0)]
                se_b = se_ring[(u, j, 1)]
                first = j == 0
                last = j == NJ - 1
                for i in range(2):
                    kv = 2 * j + i
                    sl = slice(512 * i, 512 * (i + 1))
                    h0 = DCORE * kv + 64 * (2 * p)
                    nc.tensor.matmul(
                        psA[0:64, :], v_aug[:, h0 : h0 + 64], se_a[:, sl],
                        start=(first and i == 0), stop=(last and i == 1),
                        tile_position=(0, 0), skip_group_check=True,
                    )
                    nc.tensor.matmul(
                        psB[64:128, :], v_aug[:, h0 + 64 : h0 + 128], se_b[:, sl],
                        start=(first and i == 0), stop=(last and i == 1),
                        tile_position=(0, 64), skip_group_check=True,
                    )
                nc.tensor.matmul(
                    psA[64:65, :], ones_col, se_a[:, 0:512],
                    start=first, stop=last,
                    tile_position=(0, 64), skip_group_check=True,
                )
                nc.tensor.matmul(
                    psA[96:97, :], ones_col, se_a[:, 512:1024],
                    start=first, stop=last,
                    tile_position=(0, 96), skip_group_check=True,
                )
                nc.tensor.matmul(
                    psB[0:1, :], ones_col, se_b[:, 0:512],
                    start=first, stop=last,
                    tile_position=(0, 0), skip_group_check=True,
                )
                nc.tensor.matmul(
                    psB[32:33, :], ones_col, se_b[:, 512:1024],
                    start=first, stop=last,
                    tile_position=(0, 32), skip_group_check=True,
                )

            def av_norm(u):
                """normalize unit u's AV accumulators into otn. One full-bank
                copy per head releases the PSUM accumulator immediately; the
                divide chain then runs off the SBUF scratch."""
                p, qb = divmod(u, NQB)
                qs = slice(QB * qb, QB * (qb + 1))
                psA = av_ps.pop((u, 0))
                psB = av_ps.pop((u, 1))
                scr = {}
                for a, ps_o in ((0, psA), (1, psB)):
                    scr[a] = small.tile([128, QB], F32, tag="scr", name="scr")
                    nc.vector.tensor_copy(out=scr[a], in_=ps_o)
                for a, ps_o, avsl, d0, d1 in (
                    (0, psA, slice(0, 64), 64, 96),
                    (1, psB, slice(64, 128), 0, 32),
                ):
                    s = scr[a]
                    zrow = small.tile([1, QB], F32, tag="zrow", name="zrow")
                    nc.vector.tensor_add(
                        zrow, s[d0 : d0 + 1, :], ps_o[d1 : d1 + 1, :]
                    )
                    rc = small.tile([1, QB], F32, tag="rc", name="rc")
                    nc.vector.reciprocal_approx_fast(out=rc, in_=zrow[:, :])
                    bc = small.tile([128, QB], F32, tag="bc", name="bc")
                    nc.gpsimd.partition_broadcast(bc, rc[:, :])
                    nc.vector.tensor_mul(
                        otn_sb[p][64 * a : 64 * (a + 1), qs],
                        s[avsl, :],
                        bc[avsl, :],
                    )

            def post_mtile(m, tail_idx=None):
                """post projection + output DMA for one 128-token tile. In
                the tail (after the last exp) the scores PSUM banks are free:
                borrow an R tile per m-tile so the cast/psum-recycle ladder
                has 4 half-slots in flight instead of mix's 2."""
                ms = slice(128 * m, 128 * (m + 1))
                o_t = outs.tile([128, D], BF16, tag="outp", name="outp")
                reg = None
                if tail_idx is not None:
                    reg = pss.tile(
                        [128, 1024], F32, tag=f"R{tail_idx % 2}", name="psptail"
                    )
                for nj in range(2):
                    if reg is not None:
                        ps_p = reg[:, 512 * nj : 512 * (nj + 1)]
                    else:
                        ps_p = mix.tile([128, 512], F32, tag="mix", name="psp")
                    for kp in range(2):
                        nc.tensor.matmul(
                            ps_p,
                            otn_sb[kp][:, ms],
                            wp_box["wp"][:, D * kp + 512 * nj : D * kp + 512 * (nj + 1)],
                            start=(kp == 0),
                            stop=(kp == 1),
                            skip_group_check=True,
                        )
                    # tail casts: the first tail block's casts go to the
                    # (idle, post-exp) ACT engine — the DVE FIFO still holds
                    # the final norm chains and casts queued behind them
                    # would stall the PSUM recycle. Later blocks run after
                    # the norms drain, so alternate ACT/DVE to double the
                    # cast bandwidth.
                    if tail_idx is not None and (tail_idx < 12 or nj == 0):
                        nc.scalar.copy(
                            out=o_t[:, 512 * nj : 512 * (nj + 1)], in_=ps_p
                        )
                    else:
                        nc.vector.tensor_copy(
                            out=o_t[:, 512 * nj : 512 * (nj + 1)], in_=ps_p
                        )
                nc.sync.dma_start(out=out_d[ms, :], in_=o_t)

            def post_block(qb, tail=False):
                for mi in range(QB // 128):
                    m = (QB * qb) // 128 + mi
                    post_mtile(m, tail_idx=(m if tail else None))

            # ================= emission schedule =================
            # lead-in: unit 0 scores while inputs stream in. The scores/exp
            # chain is high-priority so the static scheduler starts the exp
            # stream as soon as each k-block's projection lands, instead of
            # batching projections first.
            proj_block("k", 0, 0)
            proj_block("q", 0, 0)
            with tc.high_priority():
                scores_half(0, 0, 0)
                scores_half(0, 0, 1)
                scores_half(0, 1, 0)
                scores_half(0, 1, 1)
            proj_block("k", 0, 1)
            proj_block("k", 1, 0)
            with tc.high_priority():
                scores_half(0, 2, 0)
                scores_half(0, 2, 1)
            with tc.high_priority():
                scores_half(0, 3, 0)
                scores_half(0, 3, 1)
            proj_block("k", 1, 1)
            proj_block("k", 0, 2)
            with tc.high_priority():
                scores_half(0, 4, 0)
                scores_half(0, 4, 1)
            with tc.high_priority():
                scores_half(0, 5, 0)
                scores_half(0, 5, 1)
            proj_block("q", 0, 1)
            proj_block("k", 0, 3)
            with tc.high_priority():
                scores_half(0, 6, 0)
                scores_half(0, 6, 1)
            with tc.high_priority():
                scores_half(0, 7, 0)
                scores_half(0, 7, 1)
            proj_block("k", 1, 2)

            wp_box = {}

            def self_wp():
                t = wpool.tile([128, 2 * D], BF16, tag="wp", name="wp")
                nc.sync.dma_start(out=t, in_=wp_d[:, :])
                wp_box["wp"] = t

            # period 0: scores U1; fillers ordered DMA-independent first.
            # q02/q03 must land here (U2/U3 scores read them in periods 1/2)
            # and v0-7 must land here (U0's AV consumes all 16 tiles in
            # period 1); m-major xv packing keeps the v tiles from gating.
            def fillers_p0():
                yield lambda: proj_block("q", 1, 0)
                yield lambda: dma_x(xq_t, xqp, xq_d, 2, "xq")
                yield lambda: proj_block("q", 1, 1)
                yield lambda: dma_x(xq_t, xqp, xq_d, 3, "xq")
                yield lambda: self_wp()
                yield lambda: proj_block("k", 1, 3)
                for m in range(0, 4):
                    yield (lambda m=m: v_tile(m))
                yield lambda: dma_x(xv_t, xvp, xv_d, 3, "xv")
                for m in range(4, 8):
                    yield (lambda m=m: v_tile(m))
                yield lambda: proj_block("q", 0, 2)
                yield lambda: proj_block("q", 0, 3)

            fl = list(fillers_p0())
            fi = 0
            for j in range(NJ):
                scores_half(1, j, 0)
                scores_half(1, j, 1)
                take = (len(fl) * (j + 1)) // NJ
                while fi < take:
                    fl[fi]()
                    fi += 1

            # periods 1..6: scores U(t+1), AV U(t) (U0 lag-1 at t=1; U1's
            # catch-up runs interleaved ON MIX during period 2 so the exp
            # stream never starves); per-j emission interleaves the AV/d
            # slots between the two scores halves so neither engine stalls.
            # AV chunks for units >= 2 run one j-step early ("pattern B":
            # chunks 0,1 at step 1, chunk j+1 at step j, norm at step 7) so
            # each unit's norm completes ~one step before the next unit's
            # first AV chunk needs the PSUM banks back. Period 6 additionally
            # runs U7's AV (on mix) one j-step behind its exps; posts for
            # q-blocks 0 ride period 5's slack and 1-3 drain in the tail.
            extras = {
                1: [(lambda m=m: v_tile(m)) for m in range(8, 16)],
                4: [lambda: proj_block("q", 1, 2)],
                5: [lambda: proj_block("q", 1, 3)]
                + [(lambda m=m: post_mtile(m)) for m in range(0, 4)],
            }
            for t in range(1, 7):
                us = t + 1
                ua = 0 if t == 1 else t
                shifted = t >= 2
                ext = extras.get(t, [])
                ei = 0
                take = (len(ext) * 2) // NJ
                while ei < take:
                    ext[ei]()
                    ei += 1
                for j in range(NJ):
                    take = min(len(ext), (len(ext) * (j + 3)) // NJ)
                    while ei < take:
                        ext[ei]()
                        ei += 1
                    scores_half(us, j, 0)
                    if not shifted:
                        av_chunk(ua, j)
                    elif j == 1:
                        av_chunk(ua, 0)
                        av_chunk(ua, 1)
                        av_chunk(ua, 2)
                    elif 2 <= j <= NJ - 2:
                        av_chunk(ua, j + 1)
                    elif j == NJ - 1:
                        av_norm(ua)
                    scores_half(us, j, 1)
                    if t == 2:
                        av_chunk(1, j, on_mix=True)
                    if t == 3 and j == 3:
                        # U1's deferred norm: its mix accumulators are only
                        # needed again at t=4, and running the DVE chain here
                        # keeps it clear of the t2/t3 boundary
                        av_norm(1)
                    if t == 6 and j >= 1:
                        av_chunk(7, j - 1, on_mix=True)
                if not shifted:
                    av_norm(ua)
                if t == 6:
                    av_chunk(7, NJ - 1, on_mix=True)
                    av_norm(7)

            # tail: remaining post blocks (their second otn halves come from
            # units 5, 6 and 7), on the freed scores PSUM banks.
            post_block(1, tail=True)
            post_block(2, tail=True)
            post_block(3, tail=True)

    nc.compile()
    return nc


def _get_program(with_mask: bool):
    if with_mask not in _CACHE:
        _CACHE[with_mask] = _build(with_mask)
    return _CACHE[with_mask]


def _pack_rows(arr, bf16):
    """[8*128, F] -> [128, 8*F] tile-major (kt-major in free dim)."""
    kt, f = arr.shape[0] // 128, arr.shape[1]
    return np.ascontiguousarray(
        arr.reshape(kt, 128, f).transpose(1, 0, 2).reshape(128, kt * f)
    ).astype(bf16)


def _pack_w_page(wT_s, bias, bf16):
    """[128, 2 + 2*1024] wq/wk page: 2 leading bias columns (column p =
    bias for pair p's 128 dims), then p-major kt tiles."""
    page = np.zeros((128, WQW), np.float32)
    page[:, 0:2] = bias.reshape(2, 128).T
    for p in range(2):
        for kt in range(NKT):
            blk = wT_s[128 * kt : 128 * (kt + 1), 128 * p : 128 * (p + 1)]
            page[:, 2 + 1024 * p + 128 * kt : 2 + 1024 * p + 128 * (kt + 1)] = blk
    return np.ascontiguousarray(page).astype(bf16)


def _pack_x(x, bf16):
    """x [S, D] -> packed [128, NQB*XBW]: block nb, then kt, then token."""
    xT = x.T.astype(np.float32)  # [D, S]
    a = xT.reshape(NKT, 128, NQB, QB).transpose(1, 2, 0, 3)  # [128, nb, kt, c]
    return np.ascontiguousarray(a.reshape(128, NQB * XBW)).astype(bf16)


def _pack_xv(x, bf16):
    """x [S, D] -> [128, NQB*XBW] m-major: block nb, then 128-token tile
    within the block, then kt, then token — so v_tile(m) depends only on
    its own 1024-column quarter of the block DMA."""
    xT = x.T.astype(np.float32)  # [D, S]
    a = xT.reshape(NKT, 128, NQB, 4, 128).transpose(1, 2, 3, 0, 4)
    return np.ascontiguousarray(a.reshape(128, NQB * XBW)).astype(bf16)


def _prepare(query, key, value, mask, Wq, bq, Wk, bk, Wv, bv, Wpost, bpost,
             per_dim_scale):
    f32 = np.float32
    query = np.asarray(query, f32)
    key = np.asarray(key, f32)
    value = np.asarray(value, f32)
    mask = np.asarray(mask, f32)
    Wq = np.asarray(Wq, f32)
    bq = np.asarray(bq, f32)
    Wk = np.asarray(Wk, f32)
    bk = np.asarray(bk, f32)
    Wv = np.asarray(Wv, f32)
    bv = np.asarray(bv, f32)
    Wpost = np.asarray(Wpost, f32)
    bpost = np.asarray(bpost, f32)
    per_dim_scale = np.asarray(per_dim_scale, f32)

    r_softplus_0 = 1.442695041
    scale = (r_softplus_0 / np.sqrt(DK)) * np.log1p(np.exp(per_dim_scale))
    scale = scale.astype(f32)  # [DK]
    scale_tiled = np.tile(scale, HPC)  # [DCORE]

    with_mask = bool(np.any(mask))
    nc = _get_program(with_mask)

    bf16 = ml_dtypes.bfloat16
    in_maps = []
    for c in range(8):
        b = c // 4
        g = c % 4
        dsl = slice(DCORE * g, DCORE * (g + 1))

        wqT_s = Wq[dsl, :].T * scale_tiled[None, :]  # [D, 256] f32
        wkT_s = Wk[dsl, :].T
        wvT_s = Wv[dsl, :].T  # [D, 256]
        wpT_s = Wpost[:, dsl].T  # [256, 1024]

        m = {
            "xq": _pack_x(query[b], bf16),
            "xk": _pack_x(key[b], bf16),
            "xv": _pack_xv(value[b], bf16),
            "wq": _pack_w_page(wqT_s, bq[dsl] * scale_tiled, bf16),
            "wk": _pack_w_page(wkT_s, bk[dsl], bf16),
            "wv": _pack_rows(wvT_s, bf16),
            "wp": _pack_rows(wpT_s, bf16),
        }
        if with_mask:
            m["maskT"] = np.ascontiguousarray(mask[0, 0].T)
        in_maps.append(m)

    return nc, in_maps, bpost


def kernel(query, key, value, mask, Wq, bq, Wk, bk, Wv, bv, Wpost, bpost,
           per_dim_scale):
    global LAST_RESULTS
    nc, in_maps, bpost = _prepare(
        query, key, value, mask, Wq, bq, Wk, bk, Wv, bv, Wpost, bpost,
        per_dim_scale,
    )
    trace = os.environ.get("BASS_TRACE", "") not in ("", "0")
    if trace:
        _ensure_ntff_hook()
    res = run_bass_kernel_spmd(nc, in_maps, list(range(8)), trace=trace)
    LAST_RESULTS = res

    out = np.zeros((B, S, D), np.float32)
    for c in range(8):
        out[c // 4] += np.asarray(res.results[c]["out_p"], np.float32)
    # softmax rows sum to 1, so the value-projection bias contributes the
    # constant vector bv @ Wpost^T to every output row (folded here).
    bias = np.asarray(bpost, np.float32) + np.asarray(bv, np.float32) @ np.asarray(
        Wpost, np.float32
    ).T
    out += bias[None, None, :]
    return out


# revision 50
# speedup vs baseline: 1.0225x; 1.0027x over previous
"""Multi-head attention (B=2, S=2048, D=1024, H=16) on 8 trn2 cores.

Sharding: core c handles batch b = c//4 and heads 4g..4g+3 where g = c%4
(tensor-parallel on heads: Wq/Wk/Wv column-sharded, Wpost row-sharded).
Each core emits a partial [S, D] output; host sums the 4 partials per batch
and adds bpost.

v3 pipeline: ACT-paced stream with concurrent tiled matmuls. HW-verified
(microbenchmark + production traces): matmuls whose 32-row/col array strips
are disjoint run concurrently (4ns stagger) when adjacent in the stream.
Per (unit, j) step the PE does ~5 slot-times instead of v2's 8:
  - scores: 2 slots. Pair = (kv tile 2j, kv tile 2j+1) of the SAME head on
    opposite K=64 row halves; the i=1 member reads a row-swapped duplicate
    of kT/qT (built by DVE+GpSimd copies) so both members share one
    exp/PSUM dependency and the scheduler keeps them adjacent.
  - AV: 2 slots (both heads via M=64 col halves, no ones row).
  - softmax denominators: 1 slot (quad of concurrent M=1 ones-contractions
    into spare PSUM rows of the AV accumulator banks).
The exp stream on ACT (2 x 1107ns per step) is the pacer; projections, v
tiles and post ride the PE slack. AV chunks run one j-step early so each
unit's norm chain (one full-bank PSUM copy per head, then the divide off
SBUF scratch) finishes before the next unit needs the banks. PE is
pre-warmed with dummy matmuls so the HAM clock gate opens before real
data lands; biases ride as 2 leading columns of the wq/wk pages; xk
blocks are split across both DMA rings (they gate the lead-in exp
stream, which is emitted high-priority); the tail's post blocks borrow
the freed scores PSUM banks and cast on the idle ACT engine.
"""

import os

import numpy as np
import ml_dtypes

import concourse.bass as bass
import concourse.tile as tile
from concourse import bacc
from concourse import mybir
from concourse.bass_utils import run_bass_kernel_spmd

F32 = mybir.dt.float32
BF16 = mybir.dt.bfloat16

B, S, D, H = 2, 2048, 1024, 16
DK = D // H          # 64
HPC = 4              # heads per core
DCORE = HPC * DK     # 256 output dims per core
NKT = D // 128       # 8 contraction tiles over d_in
NMT = S // 128       # 16 token tiles
QB = 512             # query block
NQB = S // QB        # 4
NKV = S // 128       # 16 kv tiles
NJ = NKV // 2        # 8 kv-pair chunks per unit
XBW = NKT * QB       # 4096 packed x columns per 512-token block
WQW = 2 + 2 * NKT * 128   # wq/wk page: 2 bias cols + p-major kt tiles

_CACHE = {}
LAST_RESULTS = None


def _ensure_ntff_hook():
    """The agent image's antenv lacks axon_hooks; synthesize it and register
    the ctypes NTFF profiling hook so trace=True yields exec times."""
    import sys
    import types

    try:
        from antenv import axon_hooks  # noqa: F401
        return
    except ImportError:
        pass
    mod = types.ModuleType("antenv.axon_hooks")
    _state = {"hook": None}
    mod.set_axon_ntff_profile_hook = lambda h: _state.__setitem__("hook", h)
    mod.get_axon_ntff_profile_hook = lambda: _state["hook"]
    sys.modules["antenv.axon_hooks"] = mod
    import antenv

    antenv.axon_hooks = mod
    try:
        import trn_agent_boot.trn_boot as _tb

        hook = _tb._ntff_profile_via_ctypes("/opt/axon/libaxon_pjrt.so")
        mod.set_axon_ntff_profile_hook(hook)
    except Exception:
        pass


def _build(with_mask: bool):
    nc = bacc.Bacc(None, target_bir_lowering=False)

    xq_d = nc.declare_dram_parameter("xq", [128, NQB * XBW], BF16, isOutput=False)
    xk_d = nc.declare_dram_parameter("xk", [128, NQB * XBW], BF16, isOutput=False)
    xv_d = nc.declare_dram_parameter("xv", [128, NQB * XBW], BF16, isOutput=False)
    wq_d = nc.declare_dram_parameter("wq", [128, WQW], BF16, isOutput=False)
    wk_d = nc.declare_dram_parameter("wk", [128, WQW], BF16, isOutput=False)
    wv_d = nc.declare_dram_parameter("wv", [128, NKT * DCORE], BF16, isOutput=False)
    wp_d = nc.declare_dram_parameter("wp", [128, 2 * D], BF16, isOutput=False)
    maskT = None
    if with_mask:
        maskT = nc.declare_dram_parameter("maskT", [S, S], F32, isOutput=False)
    out_d = nc.declare_dram_parameter("out_p", [S, D], BF16, isOutput=True)

    with tile.TileContext(nc) as tc:
        with (
            tc.tile_pool(name="persist", bufs=1) as persist,
            tc.tile_pool(name="wpool", bufs=1) as wpool,
            tc.tile_pool(name="xkp", bufs=2) as xkp,
            tc.tile_pool(name="xqp", bufs=2) as xqp,
            tc.tile_pool(name="xvp", bufs=3) as xvp,
            tc.tile_pool(name="sexp", bufs=16) as sexp,
            tc.tile_pool(name="small", bufs=2) as small,
            tc.tile_pool(name="outs", bufs=2) as outs,
            tc.tile_pool(name="mpool", bufs=2) as mpool,
            tc.tile_pool(name="pss", bufs=1, space="PSUM") as pss,
            tc.tile_pool(name="pso", bufs=1, space="PSUM") as pso,
            tc.tile_pool(name="mix", bufs=2, space="PSUM") as mix,
        ):
            # ---- PE pre-warm (HAM clock gate) + ACT table warm ----
            warm_sb = persist.tile([128, 512], BF16, tag="warm_sb", name="warm_sb")
            nc.vector.memset(warm_sb, 0.01)
            ones_col = persist.tile([128, 1], BF16, tag="ones", name="ones")
            nc.vector.memset(ones_col, 1.0)
            warm_act = small.tile([1, 128], F32, tag="warm", name="warm")
            nc.scalar.activation(
                out=warm_act, in_=warm_sb[0:1, 0:128],
                func=mybir.ActivationFunctionType.Exp,
            )
            warm_ps = mix.tile([128, 512], F32, tag="mix", name="warmps")
            for _ in range(22):
                nc.tensor.matmul(
                    warm_ps, warm_sb[:, 0:128], warm_sb[:, :],
                    start=True, stop=True, skip_group_check=True,
                )

            # ---- input DMAs in deadline order ----
            xk_t = {}
            xq_t = {}
            xv_t = {}

            def dma_x(store, pool, src, nb, tag, split=1, eng=None):
                t = pool.tile([128, XBW], BF16, tag=tag, name=f"{tag}{nb}")
                hw = XBW // split
                for h in range(split):
                    (eng or nc.sync).dma_start(
                        out=t[:, hw * h : hw * (h + 1)],
                        in_=src[:, XBW * nb + hw * h : XBW * nb + hw * (h + 1)],
                    )
                store[nb] = t

            HWQ = 2 + NKT * 128  # bias cols + p0 tiles
            HX = XBW // 2
            wk_sb = wpool.tile([128, WQW], BF16, tag="wk", name="wk")
            nc.sync.dma_start(out=wk_sb[:, :HWQ], in_=wk_d[:, :HWQ])
            wq_sb = wpool.tile([128, WQW], BF16, tag="wq", name="wq")
            nc.scalar.dma_start(out=wq_sb[:, :HWQ], in_=wq_d[:, :HWQ])
            xk0 = xkp.tile([128, XBW], BF16, tag="xk", name="xk0")
            nc.sync.dma_start(out=xk0[:, :HX], in_=xk_d[:, :HX])
            xk_t[0] = xk0
            xq0 = xqp.tile([128, XBW], BF16, tag="xq", name="xq0")
            nc.scalar.dma_start(out=xq0[:, :HX], in_=xq_d[:, :HX])
            xq_t[0] = xq0
            nc.sync.dma_start(out=wk_sb[:, HWQ:], in_=wk_d[:, HWQ:])
            nc.scalar.dma_start(out=wq_sb[:, HWQ:], in_=wq_d[:, HWQ:])
            nc.sync.dma_start(out=xk0[:, HX:], in_=xk_d[:, HX:XBW])
            nc.scalar.dma_start(out=xq0[:, HX:], in_=xq_d[:, HX:XBW])

            # k-blocks gate the lead-in exp stream: split each remaining xk
            # block across BOTH rings (first half on sync, second on scalar
            # behind xq0) so xk1..xk3 land ~6us earlier each.
            def dma_xk_both(nb):
                t = xkp.tile([128, XBW], BF16, tag="xk", name=f"xk{nb}")
                nc.sync.dma_start(out=t[:, :HX], in_=xk_d[:, XBW * nb : XBW * nb + HX])
                nc.scalar.dma_start(
                    out=t[:, HX:], in_=xk_d[:, XBW * nb + HX : XBW * (nb + 1)]
                )
                xk_t[nb] = t

            dma_xk_both(1)
            dma_xk_both(2)
            dma_xk_both(3)
            dma_x(xv_t, xvp, xv_d, 0, "xv", split=4)
            dma_x(xq_t, xqp, xq_d, 1, "xq", eng=nc.scalar)
            wv_sb = wpool.tile([128, NKT * DCORE], BF16, tag="wv", name="wv")
            nc.scalar.dma_start(out=wv_sb, in_=wv_d[:, :])
            dma_x(xv_t, xvp, xv_d, 1, "xv", split=4)
            dma_x(xv_t, xvp, xv_d, 2, "xv", split=2, eng=nc.scalar)

            # bias cols -> f32 (tensor_scalar needs an f32 scalar operand)
            bq_sb = persist.tile([128, 2], F32, tag="bq", name="bq")
            nc.vector.tensor_copy(out=bq_sb, in_=wq_sb[:, 0:2])
            bk_sb = persist.tile([128, 2], F32, tag="bk", name="bk")
            nc.vector.tensor_copy(out=bk_sb, in_=wk_sb[:, 0:2])

            # ---- resident activations ----
            # normal layout: head-pair p, head a on rows 64a..64a+63
            # dup layout (kTd/qTd): row halves swapped (head a on the other
            # half) so the i=1 scores matmul can sit on the opposite K rows
            qT_sb = [persist.tile([128, S], BF16, tag=f"qT{p}", name=f"qT{p}") for p in range(2)]
            kT_sb = [persist.tile([128, S], BF16, tag=f"kT{p}", name=f"kT{p}") for p in range(2)]
            qTd_sb = [persist.tile([128, S], BF16, tag=f"qTd{p}", name=f"qTd{p}") for p in range(2)]
            kTd_sb = [persist.tile([128, S], BF16, tag=f"kTd{p}", name=f"kTd{p}") for p in range(2)]
            v_aug = persist.tile([128, NMT * DCORE], BF16, tag="vaug", name="vaug")
            otn_sb = [persist.tile([128, S], BF16, tag=f"otn{p}", name=f"otn{p}") for p in range(2)]

            def proj_block(which, p, nb):
                """qT/kT (+ row-swapped dup) for pair p, 512-token block nb."""
                w_sb, x_t, dst, dstd, b_sb = (
                    (wq_sb, xq_t, qT_sb, qTd_sb, bq_sb)
                    if which == "q"
                    else (wk_sb, xk_t, kT_sb, kTd_sb, bk_sb)
                )
                tb = slice(QB * nb, QB * (nb + 1))
                ps = mix.tile([128, QB], F32, tag="mix", name="psproj")
                for kt in range(NKT):
                    c0 = 2 + 1024 * p + 128 * kt
                    nc.tensor.matmul(
                        ps,
                        w_sb[:, c0 : c0 + 128],
                        x_t[nb][:, QB * kt : QB * (kt + 1)],
                        start=(kt == 0),
                        stop=(kt == NKT - 1),
                    )
                nc.vector.tensor_scalar_add(dst[p][:, tb], ps, b_sb[:, p : p + 1])
                # row-swapped duplicate, split DVE/GpSimd so the pair runs in
                # ~0.7us (it gates the i=1 scores matmuls of this block)
                nc.vector.tensor_copy(
                    out=dstd[p][64:128, tb], in_=dst[p][0:64, tb]
                )
                nc.gpsimd.tensor_copy(
                    out=dstd[p][0:64, tb], in_=dst[p][64:128, tb]
                )

            def v_tile(m):
                """one 128-token tile of v. xv is packed m-major (1024
                contiguous cols per tile) so tile m unblocks as soon as its
                quarter of the xv block lands."""
                nb, c0 = m // 4, (m % 4) * 1024
                ps_v = mix.tile([128, QB], F32, tag="mix", name="psv")
                for kt in range(NKT):
                    nc.tensor.matmul(
                        ps_v[:, :DCORE],
                        xv_t[nb][:, c0 + 128 * kt : c0 + 128 * (kt + 1)],
                        wv_sb[:, DCORE * kt : DCORE * (kt + 1)],
                        start=(kt == 0),
                        stop=(kt == NKT - 1),
                    )
                nc.vector.tensor_copy(
                    out=v_aug[:, DCORE * m : DCORE * (m + 1)],
                    in_=ps_v[:, :DCORE],
                )

            se_ring = {}  # (u, j, a) -> tile

            def scores_half(u, j, a):
                """scores+exp for unit u, kv pair (2j, 2j+1), head a.
                The two kv tiles run as a concurrent pair on opposite K=64
                row halves (i=1 via the row-swapped dup layout); both write
                halves of R[a] and depend only on exp(a) of the prior step."""
                p, qb = divmod(u, NQB)
                qs = slice(QB * qb, QB * (qb + 1))
                reg = pss.tile([128, 1024], F32, tag=f"R{a}", name=f"R{a}")
                kv0, kv1 = 2 * j, 2 * j + 1
                r0 = slice(64 * a, 64 * a + 64)
                r1 = slice(64 * (1 - a), 64 * (1 - a) + 64)
                nc.tensor.matmul(
                    reg[:, 0:512],
                    kT_sb[p][r0, 128 * kv0 : 128 * (kv0 + 1)],
                    qT_sb[p][r0, qs],
                    start=True, stop=True,
                    tile_position=(64 * a, 0),
                )
                nc.tensor.matmul(
                    reg[:, 512:1024],
                    kTd_sb[p][r1, 128 * kv1 : 128 * (kv1 + 1)],
                    qTd_sb[p][r1, qs],
                    start=True, stop=True,
                    tile_position=(64 * (1 - a), 0),
                )
                if with_mask:
                    for i, kv in ((0, kv0), (1, kv1)):
                        mt = mpool.tile([128, QB], F32, tag="mask", name="maskt")
                        nc.sync.dma_start(
                            out=mt, in_=maskT[128 * kv : 128 * (kv + 1), qs]
                        )
                        nc.vector.tensor_add(
                            reg[:, 512 * i : 512 * (i + 1)],
                            reg[:, 512 * i : 512 * (i + 1)],
                            mt,
                        )
                t = sexp.tile([128, 1024], BF16, tag=f"se{a}", name=f"se{a}")
                se_ring[(u, j, a)] = t
                nc.scalar.activation(
                    out=t, in_=reg, func=mybir.ActivationFunctionType.Exp
                )

            # AV PSUM banks per unit:
            #   psA: rows 0-63 AV head a (pos 0); row 64 d_a(i0); row 96 d_a(i1)
            #   psB: rows 64-127 AV head b (pos 64); row 0 d_b(i0); row 32 d_b(i1)
            av_ps = {}

            def av_chunk(u, j, on_mix=False):
                """AV pair slots + denominator quad for kv pair (2j, 2j+1)."""
                p, qb = divmod(u, NQB)
                if j == 0:
                    pool_, tagA, tagB = (
                        (mix, "mix", "mix") if on_mix else (pso, "psoA", "psoB")
                    )
                    av_ps[(u, 0)] = pool_.tile([128, QB], F32, tag=tagA, name="psoA")
                    av_ps[(u, 1)] = pool_.tile([128, QB], F32, tag=tagB, name="psoB")
                psA = av_ps[(u, 0)]
                psB = av_ps[(u, 1)]
                se_a = se_ring[(u, j, 0)]
                se_b = se_ring[(u, j, 1)]
                first = j == 0
                last = j == NJ - 1
                for i in range(2):
                    kv = 2 * j + i
                    sl = slice(512 * i, 512 * (i + 1))
                    h0 = DCORE * kv + 64 * (2 * p)
                    nc.tensor.matmul(
                        psA[0:64, :], v_aug[:, h0 : h0 + 64], se_a[:, sl],
                        start=(first and i == 0), stop=(last and i == 1),
                        tile_position=(0, 0), skip_group_check=True,
                    )
                    nc.tensor.matmul(
                        psB[64:128, :], v_aug[:, h0 + 64 : h0 + 128], se_b[:, sl],
                        start=(first and i == 0), stop=(last and i == 1),
                        tile_position=(0, 64), skip_group_check=True,
                    )
                nc.tensor.matmul(
                    psA[64:65, :], ones_col, se_a[:, 0:512],
                    start=first, stop=last,
                    tile_position=(0, 64), skip_group_check=True,
                )
                nc.tensor.matmul(
                    psA[96:97, :], ones_col, se_a[:, 512:1024],
                    start=first, stop=last,
                    tile_position=(0, 96), skip_group_check=True,
                )
                nc.tensor.matmul(
                    psB[0:1, :], ones_col, se_b[:, 0:512],
                    start=first, stop=last,
                    tile_position=(0, 0), skip_group_check=True,
                )
                nc.tensor.matmul(
                    psB[32:33, :], ones_col, se_b[:, 512:1024],
                    start=first, stop=last,
                    tile_position=(0, 32), skip_group_check=True,
                )

            def av_norm(u):
                """normalize unit u's AV accumulators into otn. One full-bank
                copy per head releases the PSUM accumulator immediately; the
                divide chain then runs off the SBUF scratch."""
                p, qb = divmod(u, NQB)
                qs = slice(QB * qb, QB * (qb + 1))
                psA = av_ps.pop((u, 0))
                psB = av_ps.pop((u, 1))
                scr = {}
                for a, ps_o in ((0, psA), (1, psB)):
                    scr[a] = small.tile([128, QB], F32, tag="scr", name="scr")
                    nc.vector.tensor_copy(out=scr[a], in_=ps_o)
                for a, ps_o, avsl, d0, d1 in (
                    (0, psA, slice(0, 64), 64, 96),
                    (1, psB, slice(64, 128), 0, 32),
                ):
                    s = scr[a]
                    zrow = small.tile([1, QB], F32, tag="zrow", name="zrow")
                    nc.vector.tensor_add(
                        zrow, s[d0 : d0 + 1, :], ps_o[d1 : d1 + 1, :]
                    )
                    rc = small.tile([1, QB], F32, tag="rc", name="rc")
                    nc.vector.reciprocal_approx_fast(out=rc, in_=zrow[:, :])
                    bc = small.tile([128, QB], F32, tag="bc", name="bc")
                    nc.gpsimd.partition_broadcast(bc, rc[:, :])
                    nc.vector.tensor_mul(
                        otn_sb[p][64 * a : 64 * (a + 1), qs],
                        s[avsl, :],
                        bc[avsl, :],
                    )

            def post_mtile(m, tail_idx=None):
                """post projection + output DMA for one 128-token tile. In
                the tail (after the last exp) the scores PSUM banks are free:
                borrow an R tile per m-tile so the cast/psum-recycle ladder
                has 4 half-slots in flight instead of mix's 2."""
                ms = slice(128 * m, 128 * (m + 1))
                o_t = outs.tile([128, D], BF16, tag="outp", name="outp")
                psps = None
                if tail_idx is not None:
                    k = tail_idx % 3
                    if k == 2:
                        # every 3rd m-tile borrows the freed AV banks, giving
                        # 6 PSUM half-slots in flight instead of 4
                        psps = [
                            pso.tile([128, QB], F32, tag="psoA", name="psptA"),
                            pso.tile([128, QB], F32, tag="psoB", name="psptB"),
                        ]
                    else:
                        reg = pss.tile(
                            [128, 1024], F32, tag=f"R{k}", name="psptail"
                        )
                        psps = [reg[:, 0:512], reg[:, 512:1024]]
                for nj in range(2):
                    if psps is not None:
                        ps_p = psps[nj]
                    else:
                        ps_p = mix.tile([128, 512], F32, tag="mix", name="psp")
                    for kp in range(2):
                        nc.tensor.matmul(
                            ps_p,
                            otn_sb[kp][:, ms],
                            wp_box["wp"][:, D * kp + 512 * nj : D * kp + 512 * (nj + 1)],
                            start=(kp == 0),
                            stop=(kp == 1),
                            skip_group_check=True,
                        )
                    # tail casts: the first tail block's casts go to the
                    # (idle, post-exp) ACT engine — the DVE FIFO still holds
                    # the final norm chains and casts queued behind them
                    # would stall the PSUM recycle. Later blocks run after
                    # the norms drain, so alternate ACT/DVE to double the
                    # cast bandwidth.
                    if tail_idx is not None and (tail_idx < 8 or nj == 0):
                        nc.scalar.copy(
                            out=o_t[:, 512 * nj : 512 * (nj + 1)], in_=ps_p
                        )
                    else:
                        nc.vector.tensor_copy(
                            out=o_t[:, 512 * nj : 512 * (nj + 1)], in_=ps_p
                        )
                nc.sync.dma_start(out=out_d[ms, :], in_=o_t)

            def post_block(qb, tail=False):
                for mi in range(QB // 128):
                    m = (QB * qb) // 128 + mi
                    post_mtile(m, tail_idx=(m if tail else None))

            # ================= emission schedule =================
            # lead-in: unit 0 scores while inputs stream in. The scores/exp
            # chain is high-priority so the static scheduler starts the exp
            # stream as soon as each k-block's projection lands, instead of
            # batching projections first.
            proj_block("k", 0, 0)
            proj_block("q", 0, 0)
            with tc.high_priority():
                scores_half(0, 0, 0)
                scores_half(0, 0, 1)
                scores_half(0, 1, 0)
                scores_half(0, 1, 1)
            proj_block("k", 0, 1)
            proj_block("k", 1, 0)
            with tc.high_priority():
                scores_half(0, 2, 0)
                scores_half(0, 2, 1)
            with tc.high_priority():
                scores_half(0, 3, 0)
                scores_half(0, 3, 1)
            proj_block("k", 1, 1)
            proj_block("k", 0, 2)
            with tc.high_priority():
                scores_half(0, 4, 0)
                scores_half(0, 4, 1)
            with tc.high_priority():
                scores_half(0, 5, 0)
                scores_half(0, 5, 1)
            proj_block("q", 0, 1)
            proj_block("k", 0, 3)
            with tc.high_priority():
                scores_half(0, 6, 0)
                scores_half(0, 6, 1)
            with tc.high_priority():
                scores_half(0, 7, 0)
                scores_half(0, 7, 1)
            proj_block("k", 1, 2)

            wp_box = {}

            def self_wp():
                t = wpool.tile([128, 2 * D], BF16, tag="wp", name="wp")
                nc.sync.dma_start(out=t, in_=wp_d[:, :])
                wp_box["wp"] = t

            # period 0: scores U1; fillers ordered DMA-independent first.
            # q02/q03 must land here (U2/U3 scores read them in periods 1/2)
            # and v0-7 must land here (U0's AV consumes all 16 tiles in
            # period 1); m-major xv packing keeps the v tiles from gating.
            def fillers_p0():
                yield lambda: proj_block("q", 1, 0)
                yield lambda: dma_x(xq_t, xqp, xq_d, 2, "xq")
                yield lambda: proj_block("q", 1, 1)
                yield lambda: dma_x(xq_t, xqp, xq_d, 3, "xq")
                yield lambda: self_wp()
                yield lambda: proj_block("k", 1, 3)
                for m in range(0, 4):
                    yield (lambda m=m: v_tile(m))
                yield lambda: dma_x(xv_t, xvp, xv_d, 3, "xv")
                for m in range(4, 8):
                    yield (lambda m=m: v_tile(m))
                yield lambda: proj_block("q", 0, 2)
                yield lambda: proj_block("q", 0, 3)

            fl = list(fillers_p0())
            fi = 0
            for j in range(NJ):
                scores_half(1, j, 0)
                scores_half(1, j, 1)
                take = (len(fl) * (j + 1)) // NJ
                while fi < take:
                    fl[fi]()
                    fi += 1

            # periods 1..6: scores U(t+1), AV U(t) (U0 lag-1 at t=1; U1's
            # catch-up runs interleaved ON MIX during period 2 so the exp
            # stream never starves); per-j emission interleaves the AV/d
            # slots between the two scores halves so neither engine stalls.
            # AV chunks for units >= 2 run one j-step early ("pattern B":
            # chunks 0,1 at step 1, chunk j+1 at step j, norm at step 7) so
            # each unit's norm completes ~one step before the next unit's
            # first AV chunk needs the PSUM banks back. Period 6 additionally
            # runs U7's AV (on mix) one j-step behind its exps; posts for
            # q-blocks 0 ride period 5's slack and 1-3 drain in the tail.
            extras = {
                1: [(lambda m=m: v_tile(m)) for m in range(8, 16)],
                4: [lambda: proj_block("q", 1, 2)],
                5: [lambda: proj_block("q", 1, 3)]
                + [(lambda m=m: post_mtile(m)) for m in range(0, 4)],
            }
            for t in range(1, 7):
                us = t + 1
                ua = 0 if t == 1 else t
                shifted = t >= 2
                ext = extras.get(t, [])
                ei = 0
                take = (len(ext) * 2) // NJ
                while ei < take:
                    ext[ei]()
                    ei += 1
                for j in range(NJ):
                    take = min(len(ext), (len(ext) * (j + 3)) // NJ)
                    while ei < take:
                        ext[ei]()
                        ei += 1
                    scores_half(us, j, 0)
                    if not shifted:
                        av_chunk(ua, j)
                    elif j == 1:
                        av_chunk(ua, 0)
                        av_chunk(ua, 1)
                        av_chunk(ua, 2)
                    elif 2 <= j <= NJ - 2:
                        av_chunk(ua, j + 1)
                    elif j == NJ - 1:
                        av_norm(ua)
                    scores_half(us, j, 1)
                    if t == 2:
                        av_chunk(1, j, on_mix=True)
                    if t == 3 and j == 3:
                        # U1's deferred norm: its mix accumulators are only
                        # needed again at t=4, and running the DVE chain here
                        # keeps it clear of the t2/t3 boundary
                        av_norm(1)
                    if t == 6 and j >= 1:
                        av_chunk(7, j - 1, on_mix=True)
                if not shifted:
                    av_norm(ua)
                if t == 6:
                    av_chunk(7, NJ - 1, on_mix=True)
                    av_norm(7)

            # tail: remaining post blocks (their second otn halves come from
            # units 5, 6 and 7), on the freed scores PSUM banks.
            post_block(1, tail=True)
            post_block(2, tail=True)
            post_block(3, tail=True)

    nc.compile()
    return nc


def _get_program(with_mask: bool):
    if with_mask not in _CACHE:
        _CACHE[with_mask] = _build(with_mask)
    return _CACHE[with_mask]


def _pack_rows(arr, bf16):
    """[8*128, F] -> [128, 8*F] tile-major (kt-major in free dim)."""
    kt, f = arr.shape[0] // 128, arr.shape[1]
    return np.ascontiguousarray(
        arr.reshape(kt, 128, f).transpose(1, 0, 2).reshape(128, kt * f)
    ).astype(bf16)


def _pack_w_page(wT_s, bias, bf16):
    """[128, 2 + 2*1024] wq/wk page: 2 leading bias columns (column p =
    bias for pair p's 128 dims), then p-major kt tiles."""
    page = np.zeros((128, WQW), np.float32)
    page[:, 0:2] = bias.reshape(2, 128).T
    for p in range(2):
        for kt in range(NKT):
            blk = wT_s[128 * kt : 128 * (kt + 1), 128 * p : 128 * (p + 1)]
            page[:, 2 + 1024 * p + 128 * kt : 2 + 1024 * p + 128 * (kt + 1)] = blk
    return np.ascontiguousarray(page).astype(bf16)


def _pack_x(x, bf16):
    """x [S, D] -> packed [128, NQB*XBW]: block nb, then kt, then token."""
    xT = x.T.astype(np.float32)  # [D, S]
    a = xT.reshape(NKT, 128, NQB, QB).transpose(1, 2, 0, 3)  # [128, nb, kt, c]
    return np.ascontiguousarray(a.reshape(128, NQB * XBW)).astype(bf16)


def _pack_xv(x, bf16):
    """x [S, D] -> [128, NQB*XBW] m-major: block nb, then 128-token tile
    within the block, then kt, then token — so v_tile(m) depends only on
    its own 1024-column quarter of the block DMA."""
    xT = x.T.astype(np.float32)  # [D, S]
    a = xT.reshape(NKT, 128, NQB, 4, 128).transpose(1, 2, 3, 0, 4)
    return np.ascontiguousarray(a.reshape(128, NQB * XBW)).astype(bf16)


def _prepare(query, key, value, mask, Wq, bq, Wk, bk, Wv, bv, Wpost, bpost,
             per_dim_scale):
    f32 = np.float32
    query = np.asarray(query, f32)
    key = np.asarray(key, f32)
    value = np.asarray(value, f32)
    mask = np.asarray(mask, f32)
    Wq = np.asarray(Wq, f32)
    bq = np.asarray(bq, f32)
    Wk = np.asarray(Wk, f32)
    bk = np.asarray(bk, f32)
    Wv = np.asarray(Wv, f32)
    bv = np.asarray(bv, f32)
    Wpost = np.asarray(Wpost, f32)
    bpost = np.asarray(bpost, f32)
    per_dim_scale = np.asarray(per_dim_scale, f32)

    r_softplus_0 = 1.442695041
    scale = (r_softplus_0 / np.sqrt(DK)) * np.log1p(np.exp(per_dim_scale))
    scale = scale.astype(f32)  # [DK]
    scale_tiled = np.tile(scale, HPC)  # [DCORE]

    with_mask = bool(np.any(mask))
    nc = _get_program(with_mask)

    bf16 = ml_dtypes.bfloat16
    in_maps = []
    for c in range(8):
        b = c // 4
        g = c % 4
        dsl = slice(DCORE * g, DCORE * (g + 1))

        wqT_s = Wq[dsl, :].T * scale_tiled[None, :]  # [D, 256] f32
        wkT_s = Wk[dsl, :].T
        wvT_s = Wv[dsl, :].T  # [D, 256]
        wpT_s = Wpost[:, dsl].T  # [256, 1024]

        m = {
            "xq": _pack_x(query[b], bf16),
            "xk": _pack_x(key[b], bf16),
            "xv": _pack_xv(value[b], bf16),
            "wq": _pack_w_page(wqT_s, bq[dsl] * scale_tiled, bf16),
            "wk": _pack_w_page(wkT_s, bk[dsl], bf16),
            "wv": _pack_rows(wvT_s, bf16),
            "wp": _pack_rows(wpT_s, bf16),
        }
        if with_mask:
            m["maskT"] = np.ascontiguousarray(mask[0, 0].T)
        in_maps.append(m)

    return nc, in_maps, bpost


def kernel(query, key, value, mask, Wq, bq, Wk, bk, Wv, bv, Wpost, bpost,
           per_dim_scale):
    global LAST_RESULTS
    nc, in_maps, bpost = _prepare(
        query, key, value, mask, Wq, bq, Wk, bk, Wv, bv, Wpost, bpost,
        per_dim_scale,
    )
    trace = os.environ.get("BASS_TRACE", "") not in ("", "0")
    if trace:
        _ensure_ntff_hook()
    res = run_bass_kernel_spmd(nc, in_maps, list(range(8)), trace=trace)
    LAST_RESULTS = res

    out = np.zeros((B, S, D), np.float32)
    for c in range(8):
        out[c // 4] += np.asarray(res.results[c]["out_p"], np.float32)
    # softmax rows sum to 1, so the value-projection bias contributes the
    # constant vector bv @ Wpost^T to every output row (folded here).
    bias = np.asarray(bpost, np.float32) + np.asarray(bv, np.float32) @ np.asarray(
        Wpost, np.float32
    ).T
    out += bias[None, None, :]
    return out


# revision 51
# speedup vs baseline: 1.0683x; 1.0447x over previous
"""Multi-head attention (B=2, S=2048, D=1024, H=16) on 8 trn2 cores.

Sharding: core c handles batch b = c//4 and heads 4g..4g+3 where g = c%4
(tensor-parallel on heads: Wq/Wk/Wv column-sharded, Wpost row-sharded).
Each core emits a partial [S, D] output; host sums the 4 partials per batch
and adds bpost.

v3 pipeline: ACT-paced stream with concurrent tiled matmuls. HW-verified
(microbenchmark + production traces): matmuls whose 32-row/col array strips
are disjoint run concurrently (4ns stagger) when adjacent in the stream.
Per (unit, j) step the PE does ~5 slot-times instead of v2's 8:
  - scores: 2 slots. Pair = (kv tile 2j, kv tile 2j+1) of the SAME head on
    opposite K=64 row halves; the i=1 member reads a row-swapped duplicate
    of kT/qT (built by DVE+GpSimd copies) so both members share one
    exp/PSUM dependency and the scheduler keeps them adjacent.
  - AV: 2 slots (both heads via M=64 col halves, no ones row).
  - softmax denominators: 1 slot (quad of concurrent M=1 ones-contractions
    into spare PSUM rows of the AV accumulator banks).
The exp stream on ACT (2 x 1107ns per step) is the pacer; projections, v
tiles and post ride the PE slack. AV chunks run one j-step early so each
unit's norm chain (one full-bank PSUM copy per head, then the divide off
SBUF scratch) finishes before the next unit needs the banks. PE is
pre-warmed with dummy matmuls so the HAM clock gate opens before real
data lands; biases ride as 2 leading columns of the wq/wk pages; xk
blocks are split across both DMA rings (they gate the lead-in exp
stream, which is emitted high-priority); the tail's post blocks borrow
the freed scores PSUM banks and cast on the idle ACT engine.
"""

import os

import numpy as np
import ml_dtypes

import concourse.bass as bass
import concourse.tile as tile
from concourse import bacc
from concourse import mybir
from concourse.bass_utils import run_bass_kernel_spmd

F32 = mybir.dt.float32
BF16 = mybir.dt.bfloat16

B, S, D, H = 2, 2048, 1024, 16
DK = D // H          # 64
HPC = 4              # heads per core
DCORE = HPC * DK     # 256 output dims per core
NKT = D // 128       # 8 contraction tiles over d_in
NMT = S // 128       # 16 token tiles
QB = 512             # query block
NQB = S // QB        # 4
NKV = S // 128       # 16 kv tiles
NJ = NKV // 2        # 8 kv-pair chunks per unit
XBW = NKT * QB       # 4096 packed x columns per 512-token block
WQW = 2 + 2 * NKT * 128   # wq/wk page: 2 bias cols + p-major kt tiles

_CACHE = {}
LAST_RESULTS = None


def _ensure_ntff_hook():
    """The agent image's antenv lacks axon_hooks; synthesize it and register
    the ctypes NTFF profiling hook so trace=True yields exec times."""
    import sys
    import types

    try:
        from antenv import axon_hooks  # noqa: F401
        return
    except ImportError:
        pass
    mod = types.ModuleType("antenv.axon_hooks")
    _state = {"hook": None}
    mod.set_axon_ntff_profile_hook = lambda h: _state.__setitem__("hook", h)
    mod.get_axon_ntff_profile_hook = lambda: _state["hook"]
    sys.modules["antenv.axon_hooks"] = mod
    import antenv

    antenv.axon_hooks = mod
    try:
        import trn_agent_boot.trn_boot as _tb

        hook = _tb._ntff_profile_via_ctypes("/opt/axon/libaxon_pjrt.so")
        mod.set_axon_ntff_profile_hook(hook)
    except Exception:
        pass


def _build(with_mask: bool):
    nc = bacc.Bacc(None, target_bir_lowering=False)

    xq_d = nc.declare_dram_parameter("xq", [128, NQB * XBW], BF16, isOutput=False)
    xk_d = nc.declare_dram_parameter("xk", [128, NQB * XBW], BF16, isOutput=False)
    xv_d = nc.declare_dram_parameter("xv", [128, NQB * XBW], BF16, isOutput=False)
    wq_d = nc.declare_dram_parameter("wq", [128, WQW], BF16, isOutput=False)
    wk_d = nc.declare_dram_parameter("wk", [128, WQW], BF16, isOutput=False)
    wv_d = nc.declare_dram_parameter("wv", [128, NKT * DCORE], BF16, isOutput=False)
    wp_d = nc.declare_dram_parameter("wp", [128, 2 * D], BF16, isOutput=False)
    maskT = None
    if with_mask:
        maskT = nc.declare_dram_parameter("maskT", [S, S], F32, isOutput=False)
    out_d = nc.declare_dram_parameter("out_p", [S, D], BF16, isOutput=True)

    with tile.TileContext(nc) as tc:
        with (
            tc.tile_pool(name="persist", bufs=1) as persist,
            tc.tile_pool(name="wpool", bufs=1) as wpool,
            tc.tile_pool(name="xkp", bufs=2) as xkp,
            tc.tile_pool(name="xqp", bufs=2) as xqp,
            tc.tile_pool(name="xvp", bufs=3) as xvp,
            tc.tile_pool(name="sexp", bufs=16) as sexp,
            tc.tile_pool(name="small", bufs=2) as small,
            tc.tile_pool(name="outs", bufs=3) as outs,
            tc.tile_pool(name="mpool", bufs=2) as mpool,
            tc.tile_pool(name="pss", bufs=1, space="PSUM") as pss,
            tc.tile_pool(name="pso", bufs=1, space="PSUM") as pso,
            tc.tile_pool(name="mix", bufs=2, space="PSUM") as mix,
        ):
            # ---- PE pre-warm (HAM clock gate) + ACT table warm ----
            warm_sb = persist.tile([128, 256], BF16, tag="warm_sb", name="warm_sb")
            nc.vector.memset(warm_sb, 0.01)
            ones_col = persist.tile([128, 1], BF16, tag="ones", name="ones")
            nc.vector.memset(ones_col, 1.0)
            warm_act = small.tile([1, 128], F32, tag="warm", name="warm")
            nc.scalar.activation(
                out=warm_act, in_=warm_sb[0:1, 0:128],
                func=mybir.ActivationFunctionType.Exp,
            )
            warm_ps = mix.tile([128, 512], F32, tag="mix", name="warmps")
            for _ in range(22):
                nc.tensor.matmul(
                    warm_ps[:, 0:256], warm_sb[:, 0:128], warm_sb[:, :],
                    start=True, stop=True, skip_group_check=True,
                )

            # ---- input DMAs in deadline order ----
            xk_t = {}
            xq_t = {}
            xv_t = {}

            def dma_x(store, pool, src, nb, tag, split=1, eng=None):
                t = pool.tile([128, XBW], BF16, tag=tag, name=f"{tag}{nb}")
                hw = XBW // split
                for h in range(split):
                    (eng or nc.sync).dma_start(
                        out=t[:, hw * h : hw * (h + 1)],
                        in_=src[:, XBW * nb + hw * h : XBW * nb + hw * (h + 1)],
                    )
                store[nb] = t

            HWQ = 2 + NKT * 128  # bias cols + p0 tiles
            HX = XBW // 2
            wk_sb = wpool.tile([128, WQW], BF16, tag="wk", name="wk")
            nc.sync.dma_start(out=wk_sb[:, :HWQ], in_=wk_d[:, :HWQ])
            wq_sb = wpool.tile([128, WQW], BF16, tag="wq", name="wq")
            nc.scalar.dma_start(out=wq_sb[:, :HWQ], in_=wq_d[:, :HWQ])
            xk0 = xkp.tile([128, XBW], BF16, tag="xk", name="xk0")
            nc.sync.dma_start(out=xk0[:, :HX], in_=xk_d[:, :HX])
            xk_t[0] = xk0
            xq0 = xqp.tile([128, XBW], BF16, tag="xq", name="xq0")
            nc.scalar.dma_start(out=xq0[:, :HX], in_=xq_d[:, :HX])
            xq_t[0] = xq0
            nc.sync.dma_start(out=wk_sb[:, HWQ:], in_=wk_d[:, HWQ:])
            nc.scalar.dma_start(out=wq_sb[:, HWQ:], in_=wq_d[:, HWQ:])
            nc.sync.dma_start(out=xk0[:, HX:], in_=xk_d[:, HX:XBW])
            nc.scalar.dma_start(out=xq0[:, HX:], in_=xq_d[:, HX:XBW])

            # k-blocks gate the lead-in exp stream: split each remaining xk
            # block across BOTH rings (first half on sync, second on scalar
            # behind xq0) so xk1..xk3 land ~6us earlier each.
            def dma_xk_both(nb):
                t = xkp.tile([128, XBW], BF16, tag="xk", name=f"xk{nb}")
                nc.sync.dma_start(out=t[:, :HX], in_=xk_d[:, XBW * nb : XBW * nb + HX])
                nc.scalar.dma_start(
                    out=t[:, HX:], in_=xk_d[:, XBW * nb + HX : XBW * (nb + 1)]
                )
                xk_t[nb] = t

            dma_xk_both(1)
            dma_xk_both(2)
            dma_xk_both(3)
            dma_x(xv_t, xvp, xv_d, 0, "xv", split=4)
            dma_x(xq_t, xqp, xq_d, 1, "xq", eng=nc.scalar)
            wv_sb = wpool.tile([128, NKT * DCORE], BF16, tag="wv", name="wv")
            nc.scalar.dma_start(out=wv_sb, in_=wv_d[:, :])
            dma_x(xv_t, xvp, xv_d, 1, "xv", split=4)
            dma_x(xv_t, xvp, xv_d, 2, "xv", split=2, eng=nc.scalar)

            # bias cols -> f32 (tensor_scalar needs an f32 scalar operand)
            bq_sb = persist.tile([128, 2], F32, tag="bq", name="bq")
            nc.vector.tensor_copy(out=bq_sb, in_=wq_sb[:, 0:2])
            bk_sb = persist.tile([128, 2], F32, tag="bk", name="bk")
            nc.vector.tensor_copy(out=bk_sb, in_=wk_sb[:, 0:2])

            # ---- resident activations ----
            # normal layout: head-pair p, head a on rows 64a..64a+63
            # dup layout (kTd/qTd): row halves swapped (head a on the other
            # half) so the i=1 scores matmul can sit on the opposite K rows
            qT_sb = [persist.tile([128, S], BF16, tag=f"qT{p}", name=f"qT{p}") for p in range(2)]
            kT_sb = [persist.tile([128, S], BF16, tag=f"kT{p}", name=f"kT{p}") for p in range(2)]
            qTd_sb = [persist.tile([128, S], BF16, tag=f"qTd{p}", name=f"qTd{p}") for p in range(2)]
            kTd_sb = [persist.tile([128, S], BF16, tag=f"kTd{p}", name=f"kTd{p}") for p in range(2)]
            v_aug = persist.tile([128, NMT * DCORE], BF16, tag="vaug", name="vaug")
            otn_sb = [persist.tile([128, S], BF16, tag=f"otn{p}", name=f"otn{p}") for p in range(2)]

            def proj_block(which, p, nb):
                """qT/kT (+ row-swapped dup) for pair p, 512-token block nb."""
                w_sb, x_t, dst, dstd, b_sb = (
                    (wq_sb, xq_t, qT_sb, qTd_sb, bq_sb)
                    if which == "q"
                    else (wk_sb, xk_t, kT_sb, kTd_sb, bk_sb)
                )
                tb = slice(QB * nb, QB * (nb + 1))
                ps = mix.tile([128, QB], F32, tag="mix", name="psproj")
                for kt in range(NKT):
                    c0 = 2 + 1024 * p + 128 * kt
                    nc.tensor.matmul(
                        ps,
                        w_sb[:, c0 : c0 + 128],
                        x_t[nb][:, QB * kt : QB * (kt + 1)],
                        start=(kt == 0),
                        stop=(kt == NKT - 1),
                    )
                nc.vector.tensor_scalar_add(dst[p][:, tb], ps, b_sb[:, p : p + 1])
                # row-swapped duplicate, split DVE/GpSimd so the pair runs in
                # ~0.7us (it gates the i=1 scores matmuls of this block)
                nc.vector.tensor_copy(
                    out=dstd[p][64:128, tb], in_=dst[p][0:64, tb]
                )
                nc.gpsimd.tensor_copy(
                    out=dstd[p][0:64, tb], in_=dst[p][64:128, tb]
                )

            def v_tile(m):
                """one 128-token tile of v. xv is packed m-major (1024
                contiguous cols per tile) so tile m unblocks as soon as its
                quarter of the xv block lands."""
                nb, c0 = m // 4, (m % 4) * 1024
                ps_v = mix.tile([128, QB], F32, tag="mix", name="psv")
                for kt in range(NKT):
                    nc.tensor.matmul(
                        ps_v[:, :DCORE],
                        xv_t[nb][:, c0 + 128 * kt : c0 + 128 * (kt + 1)],
                        wv_sb[:, DCORE * kt : DCORE * (kt + 1)],
                        start=(kt == 0),
                        stop=(kt == NKT - 1),
                    )
                nc.vector.tensor_copy(
                    out=v_aug[:, DCORE * m : DCORE * (m + 1)],
                    in_=ps_v[:, :DCORE],
                )

            se_ring = {}  # (u, j, a) -> tile

            def scores_half(u, j, a):
                """scores+exp for unit u, kv pair (2j, 2j+1), head a.
                The two kv tiles run as a concurrent pair on opposite K=64
                row halves (i=1 via the row-swapped dup layout); both write
                halves of R[a] and depend only on exp(a) of the prior step."""
                p, qb = divmod(u, NQB)
                qs = slice(QB * qb, QB * (qb + 1))
                reg = pss.tile([128, 1024], F32, tag=f"R{a}", name=f"R{a}")
                kv0, kv1 = 2 * j, 2 * j + 1
                r0 = slice(64 * a, 64 * a + 64)
                r1 = slice(64 * (1 - a), 64 * (1 - a) + 64)
                nc.tensor.matmul(
                    reg[:, 0:512],
                    kT_sb[p][r0, 128 * kv0 : 128 * (kv0 + 1)],
                    qT_sb[p][r0, qs],
                    start=True, stop=True,
                    tile_position=(64 * a, 0),
                )
                nc.tensor.matmul(
                    reg[:, 512:1024],
                    kTd_sb[p][r1, 128 * kv1 : 128 * (kv1 + 1)],
                    qTd_sb[p][r1, qs],
                    start=True, stop=True,
                    tile_position=(64 * (1 - a), 0),
                )
                if with_mask:
                    for i, kv in ((0, kv0), (1, kv1)):
                        mt = mpool.tile([128, QB], F32, tag="mask", name="maskt")
                        nc.sync.dma_start(
                            out=mt, in_=maskT[128 * kv : 128 * (kv + 1), qs]
                        )
                        nc.vector.tensor_add(
                            reg[:, 512 * i : 512 * (i + 1)],
                            reg[:, 512 * i : 512 * (i + 1)],
                            mt,
                        )
                t = sexp.tile([128, 1024], BF16, tag=f"se{a}", name=f"se{a}")
                se_ring[(u, j, a)] = t
                nc.scalar.activation(
                    out=t, in_=reg, func=mybir.ActivationFunctionType.Exp
                )

            # AV PSUM banks per unit:
            #   psA: rows 0-63 AV head a (pos 0); row 64 d_a(i0); row 96 d_a(i1)
            #   psB: rows 64-127 AV head b (pos 64); row 0 d_b(i0); row 32 d_b(i1)
            av_ps = {}

            def av_chunk(u, j, on_mix=False):
                """AV pair slots + denominator quad for kv pair (2j, 2j+1)."""
                p, qb = divmod(u, NQB)
                if j == 0:
                    pool_, tagA, tagB = (
                        (mix, "mix", "mix") if on_mix else (pso, "psoA", "psoB")
                    )
                    av_ps[(u, 0)] = pool_.tile([128, QB], F32, tag=tagA, name="psoA")
                    av_ps[(u, 1)] = pool_.tile([128, QB], F32, tag=tagB, name="psoB")
                psA = av_ps[(u, 0)]
                psB = av_ps[(u, 1)]
                se_a = se_ring[(u, j, 0)]
                se_b = se_ring[(u, j, 1)]
                first = j == 0
                last = j == NJ - 1
                for i in range(2):
                    kv = 2 * j + i
                    sl = slice(512 * i, 512 * (i + 1))
                    h0 = DCORE * kv + 64 * (2 * p)
                    nc.tensor.matmul(
                        psA[0:64, :], v_aug[:, h0 : h0 + 64], se_a[:, sl],
                        start=(first and i == 0), stop=(last and i == 1),
                        tile_position=(0, 0), skip_group_check=True,
                    )
                    nc.tensor.matmul(
                        psB[64:128, :], v_aug[:, h0 + 64 : h0 + 128], se_b[:, sl],
                        start=(first and i == 0), stop=(last and i == 1),
                        tile_position=(0, 64), skip_group_check=True,
                    )
                nc.tensor.matmul(
                    psA[64:65, :], ones_col, se_a[:, 0:512],
                    start=first, stop=last,
                    tile_position=(0, 64), skip_group_check=True,
                )
                nc.tensor.matmul(
                    psA[96:97, :], ones_col, se_a[:, 512:1024],
                    start=first, stop=last,
                    tile_position=(0, 96), skip_group_check=True,
                )
                nc.tensor.matmul(
                    psB[0:1, :], ones_col, se_b[:, 0:512],
                    start=first, stop=last,
                    tile_position=(0, 0), skip_group_check=True,
                )
                nc.tensor.matmul(
                    psB[32:33, :], ones_col, se_b[:, 512:1024],
                    start=first, stop=last,
                    tile_position=(0, 32), skip_group_check=True,
                )

            def av_norm(u):
                """normalize unit u's AV accumulators into otn. One full-bank
                copy per head releases the PSUM accumulator immediately; the
                divide chain then runs off the SBUF scratch."""
                p, qb = divmod(u, NQB)
                qs = slice(QB * qb, QB * (qb + 1))
                psA = av_ps.pop((u, 0))
                psB = av_ps.pop((u, 1))
                scr = {}
                for a, ps_o in ((0, psA), (1, psB)):
                    scr[a] = small.tile([128, QB], F32, tag="scr", name="scr")
                    nc.vector.tensor_copy(out=scr[a], in_=ps_o)
                for a, ps_o, avsl, d0, d1 in (
                    (0, psA, slice(0, 64), 64, 96),
                    (1, psB, slice(64, 128), 0, 32),
                ):
                    s = scr[a]
                    zrow = small.tile([1, QB], F32, tag="zrow", name="zrow")
                    nc.vector.tensor_add(
                        zrow, s[d0 : d0 + 1, :], ps_o[d1 : d1 + 1, :]
                    )
                    rc = small.tile([1, QB], F32, tag="rc", name="rc")
                    nc.vector.reciprocal_approx_fast(out=rc, in_=zrow[:, :])
                    bc = small.tile([128, QB], F32, tag="bc", name="bc")
                    nc.gpsimd.partition_broadcast(bc, rc[:, :])
                    nc.vector.tensor_mul(
                        otn_sb[p][64 * a : 64 * (a + 1), qs],
                        s[avsl, :],
                        bc[avsl, :],
                    )

            def post_mtile(m, tail_idx=None):
                """post projection + output DMA for one 128-token tile. In
                the tail (after the last exp) the scores PSUM banks are free:
                borrow an R tile per m-tile so the cast/psum-recycle ladder
                has 4 half-slots in flight instead of mix's 2."""
                ms = slice(128 * m, 128 * (m + 1))
                o_t = outs.tile([128, D], BF16, tag="outp", name="outp")
                psps = None
                if tail_idx is not None:
                    k = tail_idx % 3
                    if k == 2:
                        # every 3rd m-tile borrows the freed AV banks, giving
                        # 6 PSUM half-slots in flight instead of 4
                        psps = [
                            pso.tile([128, QB], F32, tag="psoA", name="psptA"),
                            pso.tile([128, QB], F32, tag="psoB", name="psptB"),
                        ]
                    else:
                        reg = pss.tile(
                            [128, 1024], F32, tag=f"R{k}", name="psptail"
                        )
                        psps = [reg[:, 0:512], reg[:, 512:1024]]
                for nj in range(2):
                    if psps is not None:
                        ps_p = psps[nj]
                    else:
                        ps_p = mix.tile([128, 512], F32, tag="mix", name="psp")
                    for kp in range(2):
                        nc.tensor.matmul(
                            ps_p,
                            otn_sb[kp][:, ms],
                            wp_box["wp"][:, D * kp + 512 * nj : D * kp + 512 * (nj + 1)],
                            start=(kp == 0),
                            stop=(kp == 1),
                            skip_group_check=True,
                        )
                    # tail casts: the first tail block's casts go to the
                    # (idle, post-exp) ACT engine — the DVE FIFO still holds
                    # the final norm chains and casts queued behind them
                    # would stall the PSUM recycle. Later blocks run after
                    # the norms drain, so alternate ACT/DVE to double the
                    # cast bandwidth.
                    if tail_idx is not None and (tail_idx < 8 or nj == 0):
                        nc.scalar.copy(
                            out=o_t[:, 512 * nj : 512 * (nj + 1)], in_=ps_p
                        )
                    else:
                        nc.vector.tensor_copy(
                            out=o_t[:, 512 * nj : 512 * (nj + 1)], in_=ps_p
                        )
                nc.sync.dma_start(out=out_d[ms, :], in_=o_t)

            def post_block(qb, tail=False):
                for mi in range(QB // 128):
                    m = (QB * qb) // 128 + mi
                    post_mtile(m, tail_idx=(m if tail else None))

            # ================= emission schedule =================
            # lead-in: unit 0 scores while inputs stream in. The scores/exp
            # chain is high-priority so the static scheduler starts the exp
            # stream as soon as each k-block's projection lands, instead of
            # batching projections first.
            proj_block("k", 0, 0)
            proj_block("q", 0, 0)
            with tc.high_priority():
                scores_half(0, 0, 0)
                scores_half(0, 0, 1)
                scores_half(0, 1, 0)
                scores_half(0, 1, 1)
            proj_block("k", 0, 1)
            proj_block("k", 1, 0)
            with tc.high_priority():
                scores_half(0, 2, 0)
                scores_half(0, 2, 1)
            with tc.high_priority():
                scores_half(0, 3, 0)
                scores_half(0, 3, 1)
            proj_block("k", 1, 1)
            proj_block("k", 0, 2)
            with tc.high_priority():
                scores_half(0, 4, 0)
                scores_half(0, 4, 1)
            with tc.high_priority():
                scores_half(0, 5, 0)
                scores_half(0, 5, 1)
            proj_block("q", 0, 1)
            proj_block("k", 0, 3)
            with tc.high_priority():
                scores_half(0, 6, 0)
                scores_half(0, 6, 1)
            with tc.high_priority():
                scores_half(0, 7, 0)
                scores_half(0, 7, 1)
            proj_block("k", 1, 2)

            wp_box = {}

            def self_wp():
                t = wpool.tile([128, 2 * D], BF16, tag="wp", name="wp")
                nc.sync.dma_start(out=t, in_=wp_d[:, :])
                wp_box["wp"] = t

            # period 0: scores U1; fillers ordered DMA-independent first.
            # q02/q03 must land here (U2/U3 scores read them in periods 1/2)
            # and v0-7 must land here (U0's AV consumes all 16 tiles in
            # period 1); m-major xv packing keeps the v tiles from gating.
            def fillers_p0():
                yield lambda: proj_block("q", 1, 0)
                yield lambda: dma_x(xq_t, xqp, xq_d, 2, "xq")
                yield lambda: proj_block("q", 1, 1)
                yield lambda: dma_x(xq_t, xqp, xq_d, 3, "xq")
                yield lambda: self_wp()
                yield lambda: proj_block("k", 1, 3)
                for m in range(0, 4):
                    yield (lambda m=m: v_tile(m))
                yield lambda: dma_x(xv_t, xvp, xv_d, 3, "xv")
                for m in range(4, 8):
                    yield (lambda m=m: v_tile(m))
                yield lambda: proj_block("q", 0, 2)
                yield lambda: proj_block("q", 0, 3)

            fl = list(fillers_p0())
            fi = 0
            for j in range(NJ):
                scores_half(1, j, 0)
                scores_half(1, j, 1)
                take = (len(fl) * (j + 1)) // NJ
                while fi < take:
                    fl[fi]()
                    fi += 1

            # periods 1..6: scores U(t+1), AV U(t) (U0 lag-1 at t=1; U1's
            # catch-up runs interleaved ON MIX during period 2 so the exp
            # stream never starves); per-j emission interleaves the AV/d
            # slots between the two scores halves so neither engine stalls.
            # AV chunks for units >= 2 run one j-step early ("pattern B":
            # chunks 0,1 at step 1, chunk j+1 at step j, norm at step 7) so
            # each unit's norm completes ~one step before the next unit's
            # first AV chunk needs the PSUM banks back. Period 6 additionally
            # runs U7's AV (on mix) one j-step behind its exps; posts for
            # q-blocks 0 ride period 5's slack and 1-3 drain in the tail.
            extras = {
                1: [(lambda m=m: v_tile(m)) for m in range(8, 16)],
                4: [lambda: proj_block("q", 1, 2)],
                5: [lambda: proj_block("q", 1, 3)]
                + [(lambda m=m: post_mtile(m)) for m in range(0, 4)],
            }
            for t in range(1, 7):
                us = t + 1
                ua = 0 if t == 1 else t
                shifted = t >= 2
                ext = extras.get(t, [])
                ei = 0
                take = (len(ext) * 2) // NJ
                while ei < take:
                    ext[ei]()
                    ei += 1
                for j in range(NJ):
                    take = min(len(ext), (len(ext) * (j + 3)) // NJ)
                    while ei < take:
                        ext[ei]()
                        ei += 1
                    scores_half(us, j, 0)
                    if not shifted:
                        av_chunk(ua, j)
                    elif j == 1:
                        av_chunk(ua, 0)
                        av_chunk(ua, 1)
                        av_chunk(ua, 2)
                    elif 2 <= j <= NJ - 2:
                        av_chunk(ua, j + 1)
                    elif j == NJ - 1:
                        av_norm(ua)
                    scores_half(us, j, 1)
                    if t == 2:
                        av_chunk(1, j, on_mix=True)
                    if t == 3 and j == 3:
                        # U1's deferred norm: its mix accumulators are only
                        # needed again at t=4, and running the DVE chain here
                        # keeps it clear of the t2/t3 boundary
                        av_norm(1)
                    if t == 6 and j >= 1:
                        av_chunk(7, j - 1, on_mix=True)
                if not shifted:
                    av_norm(ua)
                if t == 6:
                    av_chunk(7, NJ - 1, on_mix=True)
                    av_norm(7)

            # tail: remaining post blocks (their second otn halves come from
            # units 5, 6 and 7), on the freed scores PSUM banks.
            post_block(1, tail=True)
            post_block(2, tail=True)
            post_block(3, tail=True)

    nc.compile()
    return nc


def _get_program(with_mask: bool):
    if with_mask not in _CACHE:
        _CACHE[with_mask] = _build(with_mask)
    return _CACHE[with_mask]


def _pack_rows(arr, bf16):
    """[8*128, F] -> [128, 8*F] tile-major (kt-major in free dim)."""
    kt, f = arr.shape[0] // 128, arr.shape[1]
    return np.ascontiguousarray(
        arr.reshape(kt, 128, f).transpose(1, 0, 2).reshape(128, kt * f)
    ).astype(bf16)


def _pack_w_page(wT_s, bias, bf16):
    """[128, 2 + 2*1024] wq/wk page: 2 leading bias columns (column p =
    bias for pair p's 128 dims), then p-major kt tiles."""
    page = np.zeros((128, WQW), np.float32)
    page[:, 0:2] = bias.reshape(2, 128).T
    for p in range(2):
        for kt in range(NKT):
            blk = wT_s[128 * kt : 128 * (kt + 1), 128 * p : 128 * (p + 1)]
            page[:, 2 + 1024 * p + 128 * kt : 2 + 1024 * p + 128 * (kt + 1)] = blk
    return np.ascontiguousarray(page).astype(bf16)


def _pack_x(x, bf16):
    """x [S, D] -> packed [128, NQB*XBW]: block nb, then kt, then token."""
    xT = x.T.astype(np.float32)  # [D, S]
    a = xT.reshape(NKT, 128, NQB, QB).transpose(1, 2, 0, 3)  # [128, nb, kt, c]
    return np.ascontiguousarray(a.reshape(128, NQB * XBW)).astype(bf16)


def _pack_xv(x, bf16):
    """x [S, D] -> [128, NQB*XBW] m-major: block nb, then 128-token tile
    within the block, then kt, then token — so v_tile(m) depends only on
    its own 1024-column quarter of the block DMA."""
    xT = x.T.astype(np.float32)  # [D, S]
    a = xT.reshape(NKT, 128, NQB, 4, 128).transpose(1, 2, 3, 0, 4)
    return np.ascontiguousarray(a.reshape(128, NQB * XBW)).astype(bf16)


def _prepare(query, key, value, mask, Wq, bq, Wk, bk, Wv, bv, Wpost, bpost,
             per_dim_scale):
    f32 = np.float32
    query = np.asarray(query, f32)
    key = np.asarray(key, f32)
    value = np.asarray(value, f32)
    mask = np.asarray(mask, f32)
    Wq = np.asarray(Wq, f32)
    bq = np.asarray(bq, f32)
    Wk = np.asarray(Wk, f32)
    bk = np.asarray(bk, f32)
    Wv = np.asarray(Wv, f32)
    bv = np.asarray(bv, f32)
    Wpost = np.asarray(Wpost, f32)
    bpost = np.asarray(bpost, f32)
    per_dim_scale = np.asarray(per_dim_scale, f32)

    r_softplus_0 = 1.442695041
    scale = (r_softplus_0 / np.sqrt(DK)) * np.log1p(np.exp(per_dim_scale))
    scale = scale.astype(f32)  # [DK]
    scale_tiled = np.tile(scale, HPC)  # [DCORE]

    with_mask = bool(np.any(mask))
    nc = _get_program(with_mask)

    bf16 = ml_dtypes.bfloat16
    in_maps = []
    for c in range(8):
        b = c // 4
        g = c % 4
        dsl = slice(DCORE * g, DCORE * (g + 1))

        wqT_s = Wq[dsl, :].T * scale_tiled[None, :]  # [D, 256] f32
        wkT_s = Wk[dsl, :].T
        wvT_s = Wv[dsl, :].T  # [D, 256]
        wpT_s = Wpost[:, dsl].T  # [256, 1024]

        m = {
            "xq": _pack_x(query[b], bf16),
            "xk": _pack_x(key[b], bf16),
            "xv": _pack_xv(value[b], bf16),
            "wq": _pack_w_page(wqT_s, bq[dsl] * scale_tiled, bf16),
            "wk": _pack_w_page(wkT_s, bk[dsl], bf16),
            "wv": _pack_rows(wvT_s, bf16),
            "wp": _pack_rows(wpT_s, bf16),
        }
        if with_mask:
            m["maskT"] = np.ascontiguousarray(mask[0, 0].T)
        in_maps.append(m)

    return nc, in_maps, bpost


def kernel(query, key, value, mask, Wq, bq, Wk, bk, Wv, bv, Wpost, bpost,
           per_dim_scale):
    global LAST_RESULTS
    nc, in_maps, bpost = _prepare(
        query, key, value, mask, Wq, bq, Wk, bk, Wv, bv, Wpost, bpost,
        per_dim_scale,
    )
    trace = os.environ.get("BASS_TRACE", "") not in ("", "0")
    if trace:
        _ensure_ntff_hook()
    res = run_bass_kernel_spmd(nc, in_maps, list(range(8)), trace=trace)
    LAST_RESULTS = res

    out = np.zeros((B, S, D), np.float32)
    for c in range(8):
        out[c // 4] += np.asarray(res.results[c]["out_p"], np.float32)
    # softmax rows sum to 1, so the value-projection bias contributes the
    # constant vector bv @ Wpost^T to every output row (folded here).
    bias = np.asarray(bpost, np.float32) + np.asarray(bv, np.float32) @ np.asarray(
        Wpost, np.float32
    ).T
    out += bias[None, None, :]
    return out
